# revision 7
# baseline (speedup 1.0000x reference)
"""BestRQ layer (vq_codebook) on 8 TRN2 NeuronCores — data parallel over batch.

Math (reference semantics):
  stacked = double-unfold(raw, k=3,s=2 twice)          (B, 511, 720)
  targets = stacked @ projector                        (B, 511, 512)
  labels  = argmin_c ||targets - codebook[c]||^2 + 1, zeroed past label_lengths
  masked  = fixed-PRNG mask (key 42) on valid steps; frames under masked
            windows replaced by fixed-PRNG noise in auged_feats.

Device decomposition (per core, 4 batch items):
  * double-unfold + projector == strided conv: targets[t2] = sum_{o=0..6}
    raw[4*t2+o, :] @ P'_o where P'_o combines projector rows (CPU prep).
    Computed transposed (d on partitions) via 7 accumulating fp32r matmuls
    against a strided view of rawT.
  * scores[t2, c] = 2*t.c - ||c||^2 (argmax == argmin of distance) via
    fp32r matmuls; the -||c||^2 bias enters PSUM through a K=1 matmul.
    DVE max/max_index give top-8 values + the argmax index (first
    occurrence, matching jnp.argmin).
  * fp32r is ~1e-5-relative per dot; rows whose top-2 score gap is below
    a threshold are exactly re-solved on CPU (~1% of rows) so labels
    match the fp32 reference.
  * masked_feats: copy_predicated overwrite of auged with noise where the
    (CPU-computed, tiny) frame mask is set.
"""

import numpy as np

B, T, D = 32, 2048, 80
T2 = 511
CB = 1024
NCORES = 8
BPC = B // NCORES  # batches per core
OFFS = list(range(7))  # frame offsets per t2 window
GAP_THRESH = 0.5  # score units; fp32r err-diff rms ~2e-2 -> 25 sigma

_CACHE: dict = {}


def _cpu_jax_consts():
    """Reproduce the reference's fixed PRNG draws (key 42) on CPU jax."""
    if "noise" in _CACHE:
        return _CACHE["noise"], _CACHE["u"]
    import jax

    cpu = jax.devices("cpu")[0]
    with jax.default_device(cpu):
        import jax.numpy as jnp

        mkey = jax.random.key(42)
        km, kn = jax.random.split(mkey)
        u = np.asarray(jax.random.uniform(km, (B, T2)))
        noise = np.asarray(0.1 * jax.random.normal(kn, (B, T, D), jnp.float32))
    _CACHE["noise"] = noise
    _CACHE["u"] = u
    return noise, u


def _build_program():
    if "nc" in _CACHE:
        return _CACHE["nc"]
    import concourse.bacc as bacc
    from concourse import mybir
    from concourse.tile import TileContext

    f32 = mybir.dt.float32
    f32r = mybir.dt.float32r
    u32 = mybir.dt.uint32

    nc = bacc.Bacc()
    rawT = nc.declare_dram_parameter("rawT", [BPC, 80, 2056], f32r, isOutput=False)
    auged = nc.declare_dram_parameter("auged", [BPC, 2048, 80], f32, isOutput=False)
    noise = nc.declare_dram_parameter("noise", [BPC, 2048, 80], f32, isOutput=False)
    i32 = mybir.dt.int32
    selbc = nc.declare_dram_parameter("selbc", [BPC, 128, 16], i32, isOutput=False)
    pproj = nc.declare_dram_parameter("pproj", [80, 7, 512], f32r, isOutput=False)
    cb2t = nc.declare_dram_parameter("cb2t", [128, 4, 1024], f32r, isOutput=False)
    nbias = nc.declare_dram_parameter("nbias", [1, 1024], f32r, isOutput=False)
    ones_d = nc.declare_dram_parameter("ones_d", [1, 128], f32r, isOutput=False)
    masked = nc.declare_dram_parameter("masked", [BPC, 2048, 80], f32, isOutput=True)
    idx = nc.declare_dram_parameter("idx", [BPC, 4, 128, 1], u32, isOutput=True)
    mxv = nc.declare_dram_parameter("mxv", [BPC, 4, 128, 2], f32, isOutput=True)

    with TileContext(nc) as tc:
        with (
            tc.tile_pool(name="const", bufs=1) as const,
            tc.tile_pool(name="raw", bufs=2) as raw_pool,
            tc.tile_pool(name="tps", bufs=4, space="PSUM") as tpsum_pool,
            tc.tile_pool(name="tsb", bufs=2) as tsb_pool,
            tc.tile_pool(name="sps", bufs=2, space="PSUM") as spsum_pool,
            tc.tile_pool(name="ssb", bufs=3) as ssb_pool,
            tc.tile_pool(name="mx", bufs=4) as mx_pool,
            tc.tile_pool(name="ix", bufs=4) as ix_pool,
            tc.tile_pool(name="io", bufs=2) as io_pool,
            tc.tile_pool(name="sel", bufs=2) as sel_pool,
        ):
            pp_sb = const.tile([80, 7, 512], f32r, name="pp_sb")
            nc.sync.dma_start(out=pp_sb, in_=pproj[:])
            cb_sb = const.tile([128, 4, 1024], f32r, name="cb_sb")
            nc.sync.dma_start(out=cb_sb, in_=cb2t[:])
            nb_sb = const.tile([1, 1024], f32r, name="nb_sb")
            nc.sync.dma_start(out=nb_sb, in_=nbias[:])
            ones = const.tile([1, 128], f32r, name="ones")
            nc.sync.dma_start(out=ones, in_=ones_d[:])

            for b in range(BPC):
                rawt = raw_pool.tile([80, 2056], f32r, name="rawt")
                nc.sync.dma_start(out=rawt, in_=rawT[b])
                tsb = tsb_pool.tile([128, 4, 512], f32r, name="tsb")
                for kc in range(4):
                    tps = tpsum_pool.tile([128, 512], f32, name="tps")
                    for i, o in enumerate(OFFS):
                        nc.tensor.matmul(
                            tps[:, 0:512],
                            lhsT=pp_sb[:, i, kc * 128 : (kc + 1) * 128],
                            rhs=rawt[:, o : o + 4 * 511 + 1 : 4],
                            start=(i == 0),
                            stop=(i == len(OFFS) - 1),
                        )
                    nc.scalar.copy(out=tsb[:, kc, :], in_=tps[:, :])

                for rc in range(4):
                    M = 128
                    r0 = rc * 128
                    sps = spsum_pool.tile([128, 1024], f32, name="sps")
                    for ch in range(2):
                        # K=1 bias pass: psum <- -||c||^2
                        nc.tensor.matmul(
                            sps[:M, ch * 512 : (ch + 1) * 512],
                            lhsT=ones[:, 0:M],
                            rhs=nb_sb[:, ch * 512 : (ch + 1) * 512],
                            start=True,
                            stop=False,
                        )
                        for kc in range(4):
                            nc.tensor.matmul(
                                sps[:M, ch * 512 : (ch + 1) * 512],
                                lhsT=tsb[:, kc, r0 : r0 + M],
                                rhs=cb_sb[:, kc, ch * 512 : (ch + 1) * 512],
                                start=False,
                                stop=(kc == 3),
                            )
                    ssb = ssb_pool.tile([128, 1024], f32, name="ssb")
                    nc.scalar.copy(out=ssb[:M], in_=sps[:M])
                    mx8 = mx_pool.tile([128, 8], f32, name="mx8")
                    nc.vector.max(mx8[:M], ssb[:M])
                    ix = ix_pool.tile([128, 8], u32, name="ix")
                    nc.vector.max_index(ix[:M], mx8[:M], ssb[:M])
                    nc.sync.dma_start(out=idx[b, rc, 0:M, :], in_=ix[:M, 0:1])
                    nc.sync.dma_start(out=mxv[b, rc, 0:M, :], in_=mx8[:M, 0:2])

                aug = io_pool.tile([128, 1280], f32, name="aug")
                nc.sync.dma_start(
                    out=aug, in_=auged[b].rearrange("(p j) f -> p (j f)", p=128)
                )
                noi = io_pool.tile([128, 1280], f32, name="noi")
                nc.sync.dma_start(
                    out=noi, in_=noise[b].rearrange("(p j) f -> p (j f)", p=128)
                )
                sel = sel_pool.tile([128, 16], i32, name="sel")
                nc.sync.dma_start(out=sel, in_=selbc[b])
                selb = sel.unsqueeze(2).broadcast_to([128, 16, 80])
                # emit InstCopyPredicated with opt=False so all three APs
                # keep the same 3-D shape (the sim can't merge the step-0
                # broadcast mask while merging the contiguous operands)
                vec = nc.vector
                vec.add_instruction(
                    mybir.InstCopyPredicated(
                        name=f"I-{nc.next_id()}",
                        ins=[
                            vec.lower_ap(selb, opt=False),
                            vec.lower_ap(
                                noi.rearrange("p (j f) -> p j f", f=80), opt=False
                            ),
                        ],
                        outs=[
                            vec.lower_ap(
                                aug.rearrange("p (j f) -> p j f", f=80), opt=False
                            )
                        ],
                    )
                )
                nc.sync.dma_start(
                    out=masked[b].rearrange("(p j) f -> p (j f)", p=128), in_=aug
                )

    nc.compile()
    _CACHE["nc"] = nc
    return nc


def _prep(raw_feats, auged_feats, length, projector, codebook):
    """CPU-side prep: weight reindexing, PRNG constants, mask, sharding."""
    raw = np.ascontiguousarray(np.asarray(raw_feats, dtype=np.float32))
    auged = np.ascontiguousarray(np.asarray(auged_feats, dtype=np.float32))
    length = np.asarray(length).astype(np.int64)
    proj = np.asarray(projector, dtype=np.float32)
    cb = np.asarray(codebook, dtype=np.float32)

    noise, u = _cpu_jax_consts()

    # combined projector P'_o (o = 2*j2 + j1)
    pp = np.zeros((7, 80, 512), np.float32)
    for j1 in range(3):
        for j2 in range(3):
            pp[2 * j2 + j1] += proj[3 * j1 + j2 :: 9, :]
    pproj = np.ascontiguousarray(pp.transpose(1, 0, 2))  # (80, 7, 512)

    cb2t = np.ascontiguousarray(
        (2.0 * cb.T).reshape(4, 128, 1024).transpose(1, 0, 2)
    )  # (128, 4, 1024): [p, k, c] = 2*cb[c, k*128+p]
    nbias = (-(cb.astype(np.float64) ** 2).sum(1)).astype(np.float32)[None, :]

    ll = ((length - 3) // 2 + 1 - 3) // 2 + 1
    valid = np.arange(T2)[None, :] < ll[:, None]
    masked_steps = (u < 0.1) & valid  # (B, T2)
    mdim = masked_steps.astype(np.float32)

    # frame mask: union of windows 4*t2 + {0..6} over masked steps
    fm = np.zeros((B, T), bool)
    t2r = 4 * np.arange(T2)
    for o in range(7):
        np.logical_or.at(fm, (np.arange(B)[:, None], (t2r + o)[None, :]), masked_steps)
    sel = np.ascontiguousarray(fm.reshape(B, 128, 16).astype(np.int32))

    rawT = np.zeros((B, 80, 2056), np.float32)
    rawT[:, :, :2048] = raw.transpose(0, 2, 1)

    in_maps = []
    for m in range(NCORES):
        s = slice(m * BPC, (m + 1) * BPC)
        in_maps.append(
            {
                "rawT": rawT[s],
                "auged": auged[s],
                "noise": noise[s],
                "selbc": sel[s],
                "pproj": pproj,
                "cb2t": cb2t,
                "nbias": nbias,
                "ones_d": np.ones((1, 128), np.float32),
            }
        )
    return in_maps, valid, mdim


_STACK_F = np.arange(720) // 9
_STACK_OFF = 2 * (np.arange(720) % 3) + (np.arange(720) // 3) % 3


def _fix_near_ties(lab, gaps, raw, proj, cb):
    """Exactly re-solve rows whose device top-2 score gap is small."""
    bs, t2s = np.nonzero(gaps < GAP_THRESH)
    if len(bs) == 0:
        return 0
    frames = (4 * t2s[:, None] + _STACK_OFF[None, :]).astype(np.int64)  # (F, 720)
    stacked = raw[bs[:, None], frames, _STACK_F[None, :]].astype(np.float64)
    t = stacked @ proj.astype(np.float64)  # (F, 512)
    d = (
        (t * t).sum(1)[:, None]
        - 2.0 * (t @ cb.astype(np.float64).T)
        + (cb.astype(np.float64) ** 2).sum(1)[None, :]
    )
    lab[bs, t2s] = d.argmin(1).astype(np.int64) + 1
    return len(bs)


def kernel(raw_feats, auged_feats, length, projector, codebook):
    from concourse.bass_utils import run_bass_kernel_spmd

    in_maps, valid, mdim = _prep(raw_feats, auged_feats, length, projector, codebook)
    nc = _build_program()

    trace = bool(_CACHE.get("trace", False))
    res = run_bass_kernel_spmd(
        nc, in_maps, list(range(NCORES)), trace=trace, **_CACHE.get("run_kwargs", {})
    )
    _CACHE["last_results"] = res

    masked_feats = np.concatenate(
        [np.asarray(res.results[m]["masked"]) for m in range(NCORES)], axis=0
    )
    idx = np.concatenate(
        [np.asarray(res.results[m]["idx"]) for m in range(NCORES)], axis=0
    )  # (B, 4, 128, 1) uint32
    mxv = np.concatenate(
        [np.asarray(res.results[m]["mxv"]) for m in range(NCORES)], axis=0
    )  # (B, 4, 128, 2) f32

    lab = idx[:, :, :, 0].reshape(B, 512)[:, :T2].astype(np.int64) + 1
    gaps = (mxv[:, :, :, 0] - mxv[:, :, :, 1]).reshape(B, 512)[:, :T2]
    nfix = _fix_near_ties(
        lab,
        gaps,
        np.asarray(raw_feats, dtype=np.float32),
        np.asarray(projector, dtype=np.float32),
        np.asarray(codebook, dtype=np.float32),
    )
    _CACHE["last_nfix"] = nfix

    labels = np.where(valid, lab, 0).astype(np.int32)[None]
    return masked_feats, labels, mdim


# revision 8
# speedup vs baseline: 1.3192x; 1.3192x over previous
"""BestRQ layer (vq_codebook) on 8 TRN2 NeuronCores — data parallel over batch.

Math (reference semantics):
  stacked = double-unfold(raw, k=3,s=2 twice)          (B, 511, 720)
  targets = stacked @ projector                        (B, 511, 512)
  labels  = argmin_c ||targets - codebook[c]||^2 + 1, zeroed past label_lengths
  masked  = fixed-PRNG mask (key 42) on valid steps; frames under masked
            windows replaced by fixed-PRNG noise in auged_feats.

Device decomposition (per core, 4 batch items):
  * double-unfold + projector == strided conv: targets[t2] = sum_{o=0..6}
    raw[4*t2+o, :] @ P'_o where P'_o combines projector rows (CPU prep).
    Computed transposed (d on partitions) via 7 accumulating fp32r matmuls
    against a strided view of rawT.
  * scores[t2, c] = 2*t.c - ||c||^2 (argmax == argmin of distance) via
    fp32r matmuls; the -||c||^2 bias enters PSUM through a K=1 matmul.
    DVE max/max_index give top-8 values + the argmax index (first
    occurrence, matching jnp.argmin).
  * fp32r is ~1e-5-relative per dot; rows whose top-2 score gap is below
    a threshold are exactly re-solved on CPU (~1% of rows) so labels
    match the fp32 reference.
  * masked_feats: copy_predicated overwrite of auged with noise where the
    (CPU-computed, tiny) frame mask is set.
"""

import numpy as np
import ml_dtypes

BF16 = np.dtype(ml_dtypes.bfloat16)

B, T, D = 32, 2048, 80
T2 = 511
CB = 1024
NCORES = 8
BPC = B // NCORES  # batches per core
OFFS = list(range(7))  # frame offsets per t2 window
GAP_THRESH = 2.0  # score units; bf16 err-diff rms ~0.34 -> ~6 sigma

_CACHE: dict = {}


def _cpu_jax_consts():
    """Reproduce the reference's fixed PRNG draws (key 42) on CPU jax."""
    if "noise" in _CACHE:
        return _CACHE["noise"], _CACHE["u"]
    import jax

    cpu = jax.devices("cpu")[0]
    with jax.default_device(cpu):
        import jax.numpy as jnp

        mkey = jax.random.key(42)
        km, kn = jax.random.split(mkey)
        u = np.asarray(jax.random.uniform(km, (B, T2)))
        noise = np.asarray(0.1 * jax.random.normal(kn, (B, T, D), jnp.float32))
    _CACHE["noise"] = noise
    _CACHE["u"] = u
    return noise, u


def _build_program():
    if "nc" in _CACHE:
        return _CACHE["nc"]
    import concourse.bacc as bacc
    from concourse import mybir
    from concourse.tile import TileContext

    f32 = mybir.dt.float32
    bf16 = mybir.dt.bfloat16
    u32 = mybir.dt.uint32

    nc = bacc.Bacc()
    rawT = nc.declare_dram_parameter("rawT", [BPC, 80, 2056], bf16, isOutput=False)
    auged = nc.declare_dram_parameter("auged", [BPC, 2048, 80], f32, isOutput=False)
    noise = nc.declare_dram_parameter("noise", [BPC, 2048, 80], f32, isOutput=False)
    i32 = mybir.dt.int32
    selbc = nc.declare_dram_parameter("selbc", [BPC, 128, 16], i32, isOutput=False)
    pproj = nc.declare_dram_parameter("pproj", [80, 7, 512], bf16, isOutput=False)
    cb2t = nc.declare_dram_parameter("cb2t", [128, 4, 1024], bf16, isOutput=False)
    nbias = nc.declare_dram_parameter("nbias", [2, 1024], bf16, isOutput=False)
    ones_d = nc.declare_dram_parameter("ones_d", [2, 128], bf16, isOutput=False)
    masked = nc.declare_dram_parameter("masked", [BPC, 2048, 80], f32, isOutput=True)
    idx = nc.declare_dram_parameter("idx", [BPC, 4, 128, 1], u32, isOutput=True)
    mxv = nc.declare_dram_parameter("mxv", [BPC, 4, 128, 2], f32, isOutput=True)

    with TileContext(nc) as tc:
        with (
            tc.tile_pool(name="const", bufs=1) as const,
            tc.tile_pool(name="raw", bufs=2) as raw_pool,
            tc.tile_pool(name="tps", bufs=4, space="PSUM") as tpsum_pool,
            tc.tile_pool(name="tsb", bufs=2) as tsb_pool,
            tc.tile_pool(name="sps", bufs=2, space="PSUM") as spsum_pool,
            tc.tile_pool(name="ssb", bufs=3) as ssb_pool,
            tc.tile_pool(name="mx", bufs=4) as mx_pool,
            tc.tile_pool(name="ix", bufs=4) as ix_pool,
            tc.tile_pool(name="io", bufs=2) as io_pool,
            tc.tile_pool(name="sel", bufs=2) as sel_pool,
        ):
            pp_sb = const.tile([80, 7, 512], bf16, name="pp_sb")
            nc.sync.dma_start(out=pp_sb, in_=pproj[:])
            cb_sb = const.tile([128, 4, 1024], bf16, name="cb_sb")
            nc.sync.dma_start(out=cb_sb, in_=cb2t[:])
            nb_sb = const.tile([2, 1024], bf16, name="nb_sb")
            nc.sync.dma_start(out=nb_sb, in_=nbias[:])
            ones = const.tile([2, 128], bf16, name="ones")
            nc.sync.dma_start(out=ones, in_=ones_d[:])

            for b in range(BPC):
                rawt = raw_pool.tile([80, 2056], bf16, name="rawt")
                nc.sync.dma_start(out=rawt, in_=rawT[b])
                tsb = tsb_pool.tile([128, 4, 512], bf16, name="tsb")
                for kc in range(4):
                    tps = tpsum_pool.tile([128, 512], f32, name="tps")
                    for i, o in enumerate(OFFS):
                        nc.tensor.matmul(
                            tps[:, 0:512],
                            lhsT=pp_sb[:, i, kc * 128 : (kc + 1) * 128],
                            rhs=rawt[:, o : o + 4 * 511 + 1 : 4],
                            start=(i == 0),
                            stop=(i == len(OFFS) - 1),
                        )
                    nc.scalar.copy(out=tsb[:, kc, :], in_=tps[:, :])

                for rc in range(4):
                    M = 128
                    r0 = rc * 128
                    sps = spsum_pool.tile([128, 1024], f32, name="sps")
                    for ch in range(2):
                        # K=1 bias pass: psum <- -||c||^2
                        nc.tensor.matmul(
                            sps[:M, ch * 512 : (ch + 1) * 512],
                            lhsT=ones[:, 0:M],
                            rhs=nb_sb[:, ch * 512 : (ch + 1) * 512],
                            start=True,
                            stop=False,
                        )
                        for kc in range(4):
                            nc.tensor.matmul(
                                sps[:M, ch * 512 : (ch + 1) * 512],
                                lhsT=tsb[:, kc, r0 : r0 + M],
                                rhs=cb_sb[:, kc, ch * 512 : (ch + 1) * 512],
                                start=False,
                                stop=(kc == 3),
                            )
                    ssb = ssb_pool.tile([128, 1024], f32, name="ssb")
                    nc.scalar.copy(out=ssb[:M], in_=sps[:M])
                    mx8 = mx_pool.tile([128, 8], f32, name="mx8")
                    nc.vector.max(mx8[:M], ssb[:M])
                    ix = ix_pool.tile([128, 8], u32, name="ix")
                    nc.vector.max_index(ix[:M], mx8[:M], ssb[:M])
                    nc.sync.dma_start(out=idx[b, rc, 0:M, :], in_=ix[:M, 0:1])
                    nc.sync.dma_start(out=mxv[b, rc, 0:M, :], in_=mx8[:M, 0:2])

                aug = io_pool.tile([128, 1280], f32, name="aug")
                nc.sync.dma_start(
                    out=aug, in_=auged[b].rearrange("(p j) f -> p (j f)", p=128)
                )
                noi = io_pool.tile([128, 1280], f32, name="noi")
                nc.sync.dma_start(
                    out=noi, in_=noise[b].rearrange("(p j) f -> p (j f)", p=128)
                )
                sel = sel_pool.tile([128, 16], i32, name="sel")
                nc.sync.dma_start(out=sel, in_=selbc[b])
                selb = sel.unsqueeze(2).broadcast_to([128, 16, 80])
                # emit InstCopyPredicated with opt=False so all three APs
                # keep the same 3-D shape (the sim can't merge the step-0
                # broadcast mask while merging the contiguous operands)
                vec = nc.vector
                vec.add_instruction(
                    mybir.InstCopyPredicated(
                        name=f"I-{nc.next_id()}",
                        ins=[
                            vec.lower_ap(selb, opt=False),
                            vec.lower_ap(
                                noi.rearrange("p (j f) -> p j f", f=80), opt=False
                            ),
                        ],
                        outs=[
                            vec.lower_ap(
                                aug.rearrange("p (j f) -> p j f", f=80), opt=False
                            )
                        ],
                    )
                )
                nc.sync.dma_start(
                    out=masked[b].rearrange("(p j) f -> p (j f)", p=128), in_=aug
                )

    nc.compile()
    _CACHE["nc"] = nc
    return nc


def _prep(raw_feats, auged_feats, length, projector, codebook):
    """CPU-side prep: weight reindexing, PRNG constants, mask, sharding."""
    raw = np.ascontiguousarray(np.asarray(raw_feats, dtype=np.float32))
    auged = np.ascontiguousarray(np.asarray(auged_feats, dtype=np.float32))
    length = np.asarray(length).astype(np.int64)
    proj = np.asarray(projector, dtype=np.float32)
    cb = np.asarray(codebook, dtype=np.float32)

    noise, u = _cpu_jax_consts()

    # combined projector P'_o (o = 2*j2 + j1)
    pp = np.zeros((7, 80, 512), np.float32)
    for j1 in range(3):
        for j2 in range(3):
            pp[2 * j2 + j1] += proj[3 * j1 + j2 :: 9, :]
    pproj = np.ascontiguousarray(pp.transpose(1, 0, 2).astype(BF16))  # (80, 7, 512)

    cb2t = np.ascontiguousarray(
        (2.0 * cb.T).reshape(4, 128, 1024).transpose(1, 0, 2).astype(BF16)
    )  # (128, 4, 1024): [p, k, c] = 2*cb[c, k*128+p]
    nb = (-(cb.astype(np.float64) ** 2).sum(1)).astype(np.float32)
    nb_hi = nb.astype(BF16)
    nb_lo = (nb - nb_hi.astype(np.float32)).astype(BF16)
    nbias = np.stack([nb_hi, nb_lo])  # (2, 1024) bf16 hi+lo

    ll = ((length - 3) // 2 + 1 - 3) // 2 + 1
    valid = np.arange(T2)[None, :] < ll[:, None]
    masked_steps = (u < 0.1) & valid  # (B, T2)
    mdim = masked_steps.astype(np.float32)

    # frame mask: union of windows 4*t2 + {0..6} over masked steps
    fm = np.zeros((B, T), bool)
    t2r = 4 * np.arange(T2)
    for o in range(7):
        np.logical_or.at(fm, (np.arange(B)[:, None], (t2r + o)[None, :]), masked_steps)
    sel = np.ascontiguousarray(fm.reshape(B, 128, 16).astype(np.int32))

    rawT = np.zeros((B, 80, 2056), BF16)
    rawT[:, :, :2048] = raw.transpose(0, 2, 1).astype(BF16)

    in_maps = []
    for m in range(NCORES):
        s = slice(m * BPC, (m + 1) * BPC)
        in_maps.append(
            {
                "rawT": rawT[s],
                "auged": auged[s],
                "noise": noise[s],
                "selbc": sel[s],
                "pproj": pproj,
                "cb2t": cb2t,
                "nbias": nbias,
                "ones_d": np.ones((2, 128), BF16),
            }
        )
    return in_maps, valid, mdim


_STACK_F = np.arange(720) // 9
_STACK_OFF = 2 * (np.arange(720) % 3) + (np.arange(720) // 3) % 3


def _fix_near_ties(lab, gaps, raw, proj, cb):
    """Exactly re-solve rows whose device top-2 score gap is small."""
    bs, t2s = np.nonzero(gaps < GAP_THRESH)
    if len(bs) == 0:
        return 0
    frames = (4 * t2s[:, None] + _STACK_OFF[None, :]).astype(np.int64)  # (F, 720)
    stacked = raw[bs[:, None], frames, _STACK_F[None, :]].astype(np.float64)
    t = stacked @ proj.astype(np.float64)  # (F, 512)
    d = (
        (t * t).sum(1)[:, None]
        - 2.0 * (t @ cb.astype(np.float64).T)
        + (cb.astype(np.float64) ** 2).sum(1)[None, :]
    )
    lab[bs, t2s] = d.argmin(1).astype(np.int64) + 1
    return len(bs)


def kernel(raw_feats, auged_feats, length, projector, codebook):
    from concourse.bass_utils import run_bass_kernel_spmd

    in_maps, valid, mdim = _prep(raw_feats, auged_feats, length, projector, codebook)
    nc = _build_program()

    trace = bool(_CACHE.get("trace", False))
    res = run_bass_kernel_spmd(
        nc, in_maps, list(range(NCORES)), trace=trace, **_CACHE.get("run_kwargs", {})
    )
    _CACHE["last_results"] = res

    masked_feats = np.concatenate(
        [np.asarray(res.results[m]["masked"]) for m in range(NCORES)], axis=0
    )
    idx = np.concatenate(
        [np.asarray(res.results[m]["idx"]) for m in range(NCORES)], axis=0
    )  # (B, 4, 128, 1) uint32
    mxv = np.concatenate(
        [np.asarray(res.results[m]["mxv"]) for m in range(NCORES)], axis=0
    )  # (B, 4, 128, 2) f32

    lab = idx[:, :, :, 0].reshape(B, 512)[:, :T2].astype(np.int64) + 1
    gaps = (mxv[:, :, :, 0] - mxv[:, :, :, 1]).reshape(B, 512)[:, :T2]
    nfix = _fix_near_ties(
        lab,
        gaps,
        np.asarray(raw_feats, dtype=np.float32),
        np.asarray(projector, dtype=np.float32),
        np.asarray(codebook, dtype=np.float32),
    )
    _CACHE["last_nfix"] = nfix

    labels = np.where(valid, lab, 0).astype(np.int32)[None]
    return masked_feats, labels, mdim


# revision 9
# speedup vs baseline: 1.3496x; 1.0230x over previous
"""BestRQ layer (vq_codebook) on 8 TRN2 NeuronCores — data parallel over batch.

Math (reference semantics):
  stacked = double-unfold(raw, k=3,s=2 twice)          (B, 511, 720)
  targets = stacked @ projector                        (B, 511, 512)
  labels  = argmin_c ||targets - codebook[c]||^2 + 1, zeroed past label_lengths
  masked  = fixed-PRNG mask (key 42) on valid steps; frames under masked
            windows replaced by fixed-PRNG noise in auged_feats.

Device decomposition (per core, 4 batch items):
  * double-unfold + projector == strided conv: targets[t2] = sum_{o=0..6}
    raw[4*t2+o, :] @ P'_o where P'_o combines projector rows (CPU prep).
    Computed transposed (d on partitions) via 7 accumulating fp32r matmuls
    against a strided view of rawT.
  * scores[t2, c] = 2*t.c - ||c||^2 (argmax == argmin of distance) via
    fp32r matmuls; the -||c||^2 bias enters PSUM through a K=1 matmul.
    DVE max/max_index give top-8 values + the argmax index (first
    occurrence, matching jnp.argmin).
  * fp32r is ~1e-5-relative per dot; rows whose top-2 score gap is below
    a threshold are exactly re-solved on CPU (~1% of rows) so labels
    match the fp32 reference.
  * masked_feats: copy_predicated overwrite of auged with noise where the
    (CPU-computed, tiny) frame mask is set.
"""

import numpy as np
import ml_dtypes

BF16 = np.dtype(ml_dtypes.bfloat16)

B, T, D = 32, 2048, 80
T2 = 511
CB = 1024
NCORES = 8
BPC = B // NCORES  # batches per core
OFFS = list(range(7))  # frame offsets per t2 window
GAP_THRESH = 2.0  # score units; bf16 err-diff rms ~0.34 -> ~6 sigma

_CACHE: dict = {}


def _cpu_jax_consts():
    """Reproduce the reference's fixed PRNG draws (key 42) on CPU jax."""
    if "noise" in _CACHE:
        return _CACHE["noise"], _CACHE["u"]
    import jax

    cpu = jax.devices("cpu")[0]
    with jax.default_device(cpu):
        import jax.numpy as jnp

        mkey = jax.random.key(42)
        km, kn = jax.random.split(mkey)
        u = np.asarray(jax.random.uniform(km, (B, T2)))
        noise = np.asarray(0.1 * jax.random.normal(kn, (B, T, D), jnp.float32))
    _CACHE["noise"] = noise
    _CACHE["u"] = u
    return noise, u


def _build_program():
    if "nc" in _CACHE:
        return _CACHE["nc"]
    import concourse.bacc as bacc
    from concourse import mybir
    from concourse.tile import TileContext

    f32 = mybir.dt.float32
    bf16 = mybir.dt.bfloat16
    u32 = mybir.dt.uint32

    nc = bacc.Bacc()
    rawT = nc.declare_dram_parameter("rawT", [BPC, 80, 2056], bf16, isOutput=False)
    auged = nc.declare_dram_parameter("auged", [BPC, 2048, 80], f32, isOutput=False)
    noise = nc.declare_dram_parameter("noise", [BPC, 2048, 80], f32, isOutput=False)
    i32 = mybir.dt.int32
    selbc = nc.declare_dram_parameter("selbc", [BPC, 128, 16], i32, isOutput=False)
    pproj = nc.declare_dram_parameter("pproj", [80, 7, 512], bf16, isOutput=False)
    cb2t = nc.declare_dram_parameter("cb2t", [128, 4, 1024], bf16, isOutput=False)
    nbias = nc.declare_dram_parameter("nbias", [2, 1024], bf16, isOutput=False)
    ones_d = nc.declare_dram_parameter("ones_d", [2, 128], bf16, isOutput=False)
    masked = nc.declare_dram_parameter("masked", [BPC, 2048, 80], f32, isOutput=True)
    idx = nc.declare_dram_parameter("idx", [BPC, 4, 128, 1], u32, isOutput=True)
    mxv = nc.declare_dram_parameter("mxv", [BPC, 4, 128, 2], f32, isOutput=True)

    with TileContext(nc) as tc:
        with (
            tc.tile_pool(name="const", bufs=1) as const,
            tc.tile_pool(name="raw", bufs=2) as raw_pool,
            tc.tile_pool(name="tps", bufs=4, space="PSUM") as tpsum_pool,
            tc.tile_pool(name="tsb", bufs=2) as tsb_pool,
            tc.tile_pool(name="sps", bufs=2, space="PSUM") as spsum_pool,
            tc.tile_pool(name="ssb", bufs=3) as ssb_pool,
            tc.tile_pool(name="mx", bufs=4) as mx_pool,
            tc.tile_pool(name="ix", bufs=4) as ix_pool,
            tc.tile_pool(name="io", bufs=2) as io_pool,
            tc.tile_pool(name="sel", bufs=2) as sel_pool,
        ):
            pp_sb = const.tile([80, 7, 512], bf16, name="pp_sb")
            nc.sync.dma_start(out=pp_sb, in_=pproj[:])
            cb_sb = const.tile([128, 4, 1024], bf16, name="cb_sb")
            nc.sync.dma_start(out=cb_sb, in_=cb2t[:])
            nb_sb = const.tile([2, 1024], bf16, name="nb_sb")
            nc.sync.dma_start(out=nb_sb, in_=nbias[:])
            ones = const.tile([2, 128], bf16, name="ones")
            nc.sync.dma_start(out=ones, in_=ones_d[:])

            for b in range(BPC):
                rawt = raw_pool.tile([80, 2056], bf16, name="rawt")
                nc.sync.dma_start(out=rawt, in_=rawT[b])
                tsb = tsb_pool.tile([128, 4, 512], bf16, name="tsb")
                for kc in range(4):
                    tps = tpsum_pool.tile([128, 512], f32, name="tps")
                    for i, o in enumerate(OFFS):
                        nc.tensor.matmul(
                            tps[:, 0:512],
                            lhsT=pp_sb[:, i, kc * 128 : (kc + 1) * 128],
                            rhs=rawt[:, o : o + 4 * 511 + 1 : 4],
                            start=(i == 0),
                            stop=(i == len(OFFS) - 1),
                        )
                    nc.scalar.copy(out=tsb[:, kc, :], in_=tps[:, :])

                aug = io_pool.tile([128, 1280], f32, name="aug")
                nc.gpsimd.dma_start(
                    out=aug, in_=auged[b].rearrange("(p j) f -> p (j f)", p=128)
                )
                noi = io_pool.tile([128, 1280], f32, name="noi")
                nc.gpsimd.dma_start(
                    out=noi, in_=noise[b].rearrange("(p j) f -> p (j f)", p=128)
                )
                sel = sel_pool.tile([128, 16], i32, name="sel")
                nc.gpsimd.dma_start(out=sel, in_=selbc[b])
                selb = sel.unsqueeze(2).broadcast_to([128, 16, 80])
                # emit InstCopyPredicated with opt=False so all three APs
                # keep the same 3-D shape (the sim can't merge the step-0
                # broadcast mask while merging the contiguous operands)
                vec = nc.vector
                vec.add_instruction(
                    mybir.InstCopyPredicated(
                        name=f"I-{nc.next_id()}",
                        ins=[
                            vec.lower_ap(selb, opt=False),
                            vec.lower_ap(
                                noi.rearrange("p (j f) -> p j f", f=80), opt=False
                            ),
                        ],
                        outs=[
                            vec.lower_ap(
                                aug.rearrange("p (j f) -> p j f", f=80), opt=False
                            )
                        ],
                    )
                )
                nc.gpsimd.dma_start(
                    out=masked[b].rearrange("(p j) f -> p (j f)", p=128), in_=aug
                )

                for rc in range(4):
                    M = 128
                    r0 = rc * 128
                    sps = spsum_pool.tile([128, 1024], f32, name="sps")
                    for ch in range(2):
                        # K=2 bias pass: psum <- -||c||^2 (bf16 hi+lo)
                        nc.tensor.matmul(
                            sps[:M, ch * 512 : (ch + 1) * 512],
                            lhsT=ones[:, 0:M],
                            rhs=nb_sb[:, ch * 512 : (ch + 1) * 512],
                            start=True,
                            stop=False,
                        )
                        for kc in range(4):
                            nc.tensor.matmul(
                                sps[:M, ch * 512 : (ch + 1) * 512],
                                lhsT=tsb[:, kc, r0 : r0 + M],
                                rhs=cb_sb[:, kc, ch * 512 : (ch + 1) * 512],
                                start=False,
                                stop=(kc == 3),
                            )
                    ssb = ssb_pool.tile([128, 1024], f32, name="ssb")
                    nc.scalar.copy(out=ssb[:M], in_=sps[:M])
                    mx8 = mx_pool.tile([128, 8], f32, name="mx8")
                    nc.vector.max(mx8[:M], ssb[:M])
                    ix = ix_pool.tile([128, 8], u32, name="ix")
                    nc.vector.max_index(ix[:M], mx8[:M], ssb[:M])
                    nc.gpsimd.dma_start(out=idx[b, rc, 0:M, :], in_=ix[:M, 0:1])
                    nc.gpsimd.dma_start(out=mxv[b, rc, 0:M, :], in_=mx8[:M, 0:2])

    nc.compile()
    _CACHE["nc"] = nc
    return nc


def _prep(raw_feats, auged_feats, length, projector, codebook):
    """CPU-side prep: weight reindexing, PRNG constants, mask, sharding."""
    raw = np.ascontiguousarray(np.asarray(raw_feats, dtype=np.float32))
    auged = np.ascontiguousarray(np.asarray(auged_feats, dtype=np.float32))
    length = np.asarray(length).astype(np.int64)
    proj = np.asarray(projector, dtype=np.float32)
    cb = np.asarray(codebook, dtype=np.float32)

    noise, u = _cpu_jax_consts()

    # combined projector P'_o (o = 2*j2 + j1)
    pp = np.zeros((7, 80, 512), np.float32)
    for j1 in range(3):
        for j2 in range(3):
            pp[2 * j2 + j1] += proj[3 * j1 + j2 :: 9, :]
    pproj = np.ascontiguousarray(pp.transpose(1, 0, 2).astype(BF16))  # (80, 7, 512)

    cb2t = np.ascontiguousarray(
        (2.0 * cb.T).reshape(4, 128, 1024).transpose(1, 0, 2).astype(BF16)
    )  # (128, 4, 1024): [p, k, c] = 2*cb[c, k*128+p]
    nb = (-(cb.astype(np.float64) ** 2).sum(1)).astype(np.float32)
    nb_hi = nb.astype(BF16)
    nb_lo = (nb - nb_hi.astype(np.float32)).astype(BF16)
    nbias = np.stack([nb_hi, nb_lo])  # (2, 1024) bf16 hi+lo

    ll = ((length - 3) // 2 + 1 - 3) // 2 + 1
    valid = np.arange(T2)[None, :] < ll[:, None]
    masked_steps = (u < 0.1) & valid  # (B, T2)
    mdim = masked_steps.astype(np.float32)

    # frame mask: union of windows 4*t2 + {0..6} over masked steps
    fm = np.zeros((B, T), bool)
    t2r = 4 * np.arange(T2)
    for o in range(7):
        np.logical_or.at(fm, (np.arange(B)[:, None], (t2r + o)[None, :]), masked_steps)
    sel = np.ascontiguousarray(fm.reshape(B, 128, 16).astype(np.int32))

    rawT = np.zeros((B, 80, 2056), BF16)
    rawT[:, :, :2048] = raw.transpose(0, 2, 1).astype(BF16)

    in_maps = []
    for m in range(NCORES):
        s = slice(m * BPC, (m + 1) * BPC)
        in_maps.append(
            {
                "rawT": rawT[s],
                "auged": auged[s],
                "noise": noise[s],
                "selbc": sel[s],
                "pproj": pproj,
                "cb2t": cb2t,
                "nbias": nbias,
                "ones_d": np.ones((2, 128), BF16),
            }
        )
    return in_maps, valid, mdim


_STACK_F = np.arange(720) // 9
_STACK_OFF = 2 * (np.arange(720) % 3) + (np.arange(720) // 3) % 3


def _fix_near_ties(lab, gaps, raw, proj, cb):
    """Exactly re-solve rows whose device top-2 score gap is small."""
    bs, t2s = np.nonzero(gaps < GAP_THRESH)
    if len(bs) == 0:
        return 0
    frames = (4 * t2s[:, None] + _STACK_OFF[None, :]).astype(np.int64)  # (F, 720)
    stacked = raw[bs[:, None], frames, _STACK_F[None, :]].astype(np.float64)
    t = stacked @ proj.astype(np.float64)  # (F, 512)
    d = (
        (t * t).sum(1)[:, None]
        - 2.0 * (t @ cb.astype(np.float64).T)
        + (cb.astype(np.float64) ** 2).sum(1)[None, :]
    )
    lab[bs, t2s] = d.argmin(1).astype(np.int64) + 1
    return len(bs)


def kernel(raw_feats, auged_feats, length, projector, codebook):
    from concourse.bass_utils import run_bass_kernel_spmd

    in_maps, valid, mdim = _prep(raw_feats, auged_feats, length, projector, codebook)
    nc = _build_program()

    trace = bool(_CACHE.get("trace", False))
    res = run_bass_kernel_spmd(
        nc, in_maps, list(range(NCORES)), trace=trace, **_CACHE.get("run_kwargs", {})
    )
    _CACHE["last_results"] = res

    masked_feats = np.concatenate(
        [np.asarray(res.results[m]["masked"]) for m in range(NCORES)], axis=0
    )
    idx = np.concatenate(
        [np.asarray(res.results[m]["idx"]) for m in range(NCORES)], axis=0
    )  # (B, 4, 128, 1) uint32
    mxv = np.concatenate(
        [np.asarray(res.results[m]["mxv"]) for m in range(NCORES)], axis=0
    )  # (B, 4, 128, 2) f32

    lab = idx[:, :, :, 0].reshape(B, 512)[:, :T2].astype(np.int64) + 1
    gaps = (mxv[:, :, :, 0] - mxv[:, :, :, 1]).reshape(B, 512)[:, :T2]
    nfix = _fix_near_ties(
        lab,
        gaps,
        np.asarray(raw_feats, dtype=np.float32),
        np.asarray(projector, dtype=np.float32),
        np.asarray(codebook, dtype=np.float32),
    )
    _CACHE["last_nfix"] = nfix

    labels = np.where(valid, lab, 0).astype(np.int32)[None]
    return masked_feats, labels, mdim


# revision 10
# speedup vs baseline: 1.5348x; 1.1373x over previous
"""BestRQ layer (vq_codebook) on 8 TRN2 NeuronCores — data parallel over batch.

Math (reference semantics):
  stacked = double-unfold(raw, k=3,s=2 twice)          (B, 511, 720)
  targets = stacked @ projector                        (B, 511, 512)
  labels  = argmin_c ||targets - codebook[c]||^2 + 1, zeroed past label_lengths
  masked  = fixed-PRNG mask (key 42) on valid steps; frames under masked
            windows replaced by fixed-PRNG noise in auged_feats.

Device decomposition (per core, 4 batch items):
  * double-unfold + projector == strided conv: targets[t2] = sum_{o=0..6}
    raw[4*t2+o, :] @ P'_o where P'_o combines projector rows (CPU prep).
    Computed transposed (d on partitions) via 7 accumulating fp32r matmuls
    against a strided view of rawT.
  * scores[t2, c] = 2*t.c - ||c||^2 (argmax == argmin of distance) via
    fp32r matmuls; the -||c||^2 bias enters PSUM through a K=1 matmul.
    DVE max/max_index give top-8 values + the argmax index (first
    occurrence, matching jnp.argmin).
  * fp32r is ~1e-5-relative per dot; rows whose top-2 score gap is below
    a threshold are exactly re-solved on CPU (~1% of rows) so labels
    match the fp32 reference.
  * masked_feats: copy_predicated overwrite of auged with noise where the
    (CPU-computed, tiny) frame mask is set.
"""

import numpy as np
import ml_dtypes

BF16 = np.dtype(ml_dtypes.bfloat16)

B, T, D = 32, 2048, 80
T2 = 511
CB = 1024
NCORES = 8
BPC = B // NCORES  # batches per core
OFFS = list(range(7))  # frame offsets per t2 window
GAP_THRESH = 2.0  # score units; bf16 err-diff rms ~0.34 -> ~6 sigma

_CACHE: dict = {}


def _cpu_jax_consts():
    """Reproduce the reference's fixed PRNG draws (key 42) on CPU jax."""
    if "noise" in _CACHE:
        return _CACHE["noise"], _CACHE["u"]
    import jax

    cpu = jax.devices("cpu")[0]
    with jax.default_device(cpu):
        import jax.numpy as jnp

        mkey = jax.random.key(42)
        km, kn = jax.random.split(mkey)
        u = np.asarray(jax.random.uniform(km, (B, T2)))
        noise = np.asarray(0.1 * jax.random.normal(kn, (B, T, D), jnp.float32))
    _CACHE["noise"] = noise
    _CACHE["u"] = u
    return noise, u


def _build_program():
    if "nc" in _CACHE:
        return _CACHE["nc"]
    import concourse.bacc as bacc
    from concourse import mybir
    from concourse.tile import TileContext

    f32 = mybir.dt.float32
    bf16 = mybir.dt.bfloat16
    u32 = mybir.dt.uint32

    nc = bacc.Bacc()
    rawT = nc.declare_dram_parameter("rawT", [BPC, 128, 2056], bf16, isOutput=False)
    auged = nc.declare_dram_parameter("auged", [BPC, 2048, 80], f32, isOutput=False)
    noise = nc.declare_dram_parameter("noise", [BPC, 2048, 80], f32, isOutput=False)
    i32 = mybir.dt.int32
    selbc = nc.declare_dram_parameter("selbc", [BPC, 128, 16], i32, isOutput=False)
    pproj = nc.declare_dram_parameter("pproj", [128, 7, 512], bf16, isOutput=False)
    cb2t = nc.declare_dram_parameter("cb2t", [128, 4, 1024], bf16, isOutput=False)
    nbias = nc.declare_dram_parameter("nbias", [128, 1024], bf16, isOutput=False)
    ones_d = nc.declare_dram_parameter("ones_d", [128, 128], bf16, isOutput=False)
    masked = nc.declare_dram_parameter("masked", [BPC, 2048, 80], f32, isOutput=True)
    idx = nc.declare_dram_parameter("idx", [BPC, 4, 128, 1], u32, isOutput=True)
    mxv = nc.declare_dram_parameter("mxv", [BPC, 4, 128, 2], f32, isOutput=True)

    with TileContext(nc) as tc:
        with (
            tc.tile_pool(name="const", bufs=1) as const,
            tc.tile_pool(name="raw", bufs=2) as raw_pool,
            tc.tile_pool(name="tps", bufs=4, space="PSUM") as tpsum_pool,
            tc.tile_pool(name="tsb", bufs=2) as tsb_pool,
            tc.tile_pool(name="sps", bufs=2, space="PSUM") as spsum_pool,
            tc.tile_pool(name="ssb", bufs=3) as ssb_pool,
            tc.tile_pool(name="mx", bufs=4) as mx_pool,
            tc.tile_pool(name="ix", bufs=4) as ix_pool,
            tc.tile_pool(name="io", bufs=2) as io_pool,
            tc.tile_pool(name="sel", bufs=2) as sel_pool,
        ):
            pp_sb = const.tile([128, 7, 512], bf16, name="pp_sb")
            nc.sync.dma_start(out=pp_sb, in_=pproj[:])
            cb_sb = const.tile([128, 4, 1024], bf16, name="cb_sb")
            nb_sb = const.tile([128, 1024], bf16, name="nb_sb")
            ones = const.tile([128, 128], bf16, name="ones")

            for b in range(BPC):
                rawt = raw_pool.tile([128, 2056], bf16, name="rawt")
                nc.sync.dma_start(out=rawt, in_=rawT[b])
                if b == 0:
                    # constants needed only from the scores phase onward
                    nc.sync.dma_start(out=cb_sb, in_=cb2t[:])
                    nc.sync.dma_start(out=nb_sb, in_=nbias[:])
                    nc.sync.dma_start(out=ones, in_=ones_d[:])
                tsb = tsb_pool.tile([128, 4, 512], bf16, name="tsb")
                for kc in range(4):
                    tps = tpsum_pool.tile([128, 512], f32, name="tps")
                    for i, o in enumerate(OFFS):
                        nc.tensor.matmul(
                            tps[:, 0:512],
                            lhsT=pp_sb[:, i, kc * 128 : (kc + 1) * 128],
                            rhs=rawt[:, o : o + 4 * 511 + 1 : 4],
                            start=(i == 0),
                            stop=(i == len(OFFS) - 1),
                        )
                    nc.scalar.copy(out=tsb[:, kc, :], in_=tps[:, :])

                aug = io_pool.tile([128, 1280], f32, name="aug")
                nc.gpsimd.dma_start(
                    out=aug, in_=auged[b].rearrange("(p j) f -> p (j f)", p=128)
                )
                noi = io_pool.tile([128, 1280], f32, name="noi")
                nc.gpsimd.dma_start(
                    out=noi, in_=noise[b].rearrange("(p j) f -> p (j f)", p=128)
                )
                sel = sel_pool.tile([128, 16], i32, name="sel")
                nc.gpsimd.dma_start(out=sel, in_=selbc[b])
                selb = sel.unsqueeze(2).broadcast_to([128, 16, 80])
                # emit InstCopyPredicated with opt=False so all three APs
                # keep the same 3-D shape (the sim can't merge the step-0
                # broadcast mask while merging the contiguous operands)
                vec = nc.vector
                vec.add_instruction(
                    mybir.InstCopyPredicated(
                        name=f"I-{nc.next_id()}",
                        ins=[
                            vec.lower_ap(selb, opt=False),
                            vec.lower_ap(
                                noi.rearrange("p (j f) -> p j f", f=80), opt=False
                            ),
                        ],
                        outs=[
                            vec.lower_ap(
                                aug.rearrange("p (j f) -> p j f", f=80), opt=False
                            )
                        ],
                    )
                )
                nc.gpsimd.dma_start(
                    out=masked[b].rearrange("(p j) f -> p (j f)", p=128), in_=aug
                )

                for rc in range(4):
                    M = 128
                    r0 = rc * 128
                    sps = spsum_pool.tile([128, 1024], f32, name="sps")
                    for ch in range(2):
                        # K=2 bias pass: psum <- -||c||^2 (bf16 hi+lo)
                        nc.tensor.matmul(
                            sps[:M, ch * 512 : (ch + 1) * 512],
                            lhsT=ones[:, 0:M],
                            rhs=nb_sb[:, ch * 512 : (ch + 1) * 512],
                            start=True,
                            stop=False,
                        )
                        for kc in range(4):
                            nc.tensor.matmul(
                                sps[:M, ch * 512 : (ch + 1) * 512],
                                lhsT=tsb[:, kc, r0 : r0 + M],
                                rhs=cb_sb[:, kc, ch * 512 : (ch + 1) * 512],
                                start=False,
                                stop=(kc == 3),
                            )
                    ssb = ssb_pool.tile([128, 1024], f32, name="ssb")
                    nc.scalar.copy(out=ssb[:M], in_=sps[:M])
                    mx8 = mx_pool.tile([128, 8], f32, name="mx8")
                    nc.vector.max(mx8[:M], ssb[:M])
                    ix = ix_pool.tile([128, 8], u32, name="ix")
                    nc.vector.max_index(ix[:M], mx8[:M], ssb[:M])
                    nc.gpsimd.dma_start(out=idx[b, rc, 0:M, :], in_=ix[:M, 0:1])
                    nc.gpsimd.dma_start(out=mxv[b, rc, 0:M, :], in_=mx8[:M, 0:2])

    nc.compile()
    _CACHE["nc"] = nc
    return nc


def _prep(raw_feats, auged_feats, length, projector, codebook):
    """CPU-side prep: weight reindexing, PRNG constants, mask, sharding."""
    raw = np.ascontiguousarray(np.asarray(raw_feats, dtype=np.float32))
    auged = np.ascontiguousarray(np.asarray(auged_feats, dtype=np.float32))
    length = np.asarray(length).astype(np.int64)
    proj = np.asarray(projector, dtype=np.float32)
    cb = np.asarray(codebook, dtype=np.float32)

    noise, u = _cpu_jax_consts()

    # combined projector P'_o (o = 2*j2 + j1)
    pp = np.zeros((7, 80, 512), np.float32)
    for j1 in range(3):
        for j2 in range(3):
            pp[2 * j2 + j1] += proj[3 * j1 + j2 :: 9, :]
    pproj = np.zeros((128, 7, 512), BF16)
    pproj[:80] = pp.transpose(1, 0, 2).astype(BF16)

    cb2t = np.ascontiguousarray(
        (2.0 * cb.T).reshape(4, 128, 1024).transpose(1, 0, 2).astype(BF16)
    )  # (128, 4, 1024): [p, k, c] = 2*cb[c, k*128+p]
    nb = (-(cb.astype(np.float64) ** 2).sum(1)).astype(np.float32)
    nbias = np.zeros((128, 1024), BF16)
    nbias[0] = nb.astype(BF16)
    nbias[1] = (nb - nbias[0].astype(np.float32)).astype(BF16)

    ll = ((length - 3) // 2 + 1 - 3) // 2 + 1
    valid = np.arange(T2)[None, :] < ll[:, None]
    masked_steps = (u < 0.1) & valid  # (B, T2)
    mdim = masked_steps.astype(np.float32)

    # frame mask: union of windows 4*t2 + {0..6} over masked steps
    fm = np.zeros((B, T), bool)
    t2r = 4 * np.arange(T2)
    for o in range(7):
        np.logical_or.at(fm, (np.arange(B)[:, None], (t2r + o)[None, :]), masked_steps)
    sel = np.ascontiguousarray(fm.reshape(B, 128, 16).astype(np.int32))

    rawT = np.zeros((B, 128, 2056), BF16)
    rawT[:, :80, :2048] = raw.transpose(0, 2, 1).astype(BF16)

    in_maps = []
    for m in range(NCORES):
        s = slice(m * BPC, (m + 1) * BPC)
        in_maps.append(
            {
                "rawT": rawT[s],
                "auged": auged[s],
                "noise": noise[s],
                "selbc": sel[s],
                "pproj": pproj,
                "cb2t": cb2t,
                "nbias": nbias,
                "ones_d": _CACHE.setdefault("ones128", _mk_ones()),
            }
        )
    return in_maps, valid, mdim


def _mk_ones():
    o = np.zeros((128, 128), BF16)
    o[0:2] = 1.0
    return o


_STACK_F = np.arange(720) // 9
_STACK_OFF = 2 * (np.arange(720) % 3) + (np.arange(720) // 3) % 3


def _fix_near_ties(lab, gaps, raw, proj, cb):
    """Exactly re-solve rows whose device top-2 score gap is small."""
    bs, t2s = np.nonzero(gaps < GAP_THRESH)
    if len(bs) == 0:
        return 0
    frames = (4 * t2s[:, None] + _STACK_OFF[None, :]).astype(np.int64)  # (F, 720)
    stacked = raw[bs[:, None], frames, _STACK_F[None, :]].astype(np.float64)
    t = stacked @ proj.astype(np.float64)  # (F, 512)
    d = (
        (t * t).sum(1)[:, None]
        - 2.0 * (t @ cb.astype(np.float64).T)
        + (cb.astype(np.float64) ** 2).sum(1)[None, :]
    )
    lab[bs, t2s] = d.argmin(1).astype(np.int64) + 1
    return len(bs)


def kernel(raw_feats, auged_feats, length, projector, codebook):
    from concourse.bass_utils import run_bass_kernel_spmd

    in_maps, valid, mdim = _prep(raw_feats, auged_feats, length, projector, codebook)
    nc = _build_program()

    trace = bool(_CACHE.get("trace", False))
    res = run_bass_kernel_spmd(
        nc, in_maps, list(range(NCORES)), trace=trace, **_CACHE.get("run_kwargs", {})
    )
    _CACHE["last_results"] = res

    masked_feats = np.concatenate(
        [np.asarray(res.results[m]["masked"]) for m in range(NCORES)], axis=0
    )
    idx = np.concatenate(
        [np.asarray(res.results[m]["idx"]) for m in range(NCORES)], axis=0
    )  # (B, 4, 128, 1) uint32
    mxv = np.concatenate(
        [np.asarray(res.results[m]["mxv"]) for m in range(NCORES)], axis=0
    )  # (B, 4, 128, 2) f32

    lab = idx[:, :, :, 0].reshape(B, 512)[:, :T2].astype(np.int64) + 1
    gaps = (mxv[:, :, :, 0] - mxv[:, :, :, 1]).reshape(B, 512)[:, :T2]
    nfix = _fix_near_ties(
        lab,
        gaps,
        np.asarray(raw_feats, dtype=np.float32),
        np.asarray(projector, dtype=np.float32),
        np.asarray(codebook, dtype=np.float32),
    )
    _CACHE["last_nfix"] = nfix

    labels = np.where(valid, lab, 0).astype(np.int32)[None]
    return masked_feats, labels, mdim


# revision 11
# speedup vs baseline: 2.1681x; 1.4126x over previous
"""BestRQ layer (vq_codebook) on 8 TRN2 NeuronCores — data parallel over batch.

Math (reference semantics):
  stacked = double-unfold(raw, k=3,s=2 twice)          (B, 511, 720)
  targets = stacked @ projector                        (B, 511, 512)
  labels  = argmin_c ||targets - codebook[c]||^2 + 1, zeroed past label_lengths
  masked  = fixed-PRNG mask (key 42) on valid steps; frames under masked
            windows replaced by fixed-PRNG noise in auged_feats.

Device decomposition (per core, 4 batch items):
  * double-unfold + projector == strided conv: targets[t2] = sum_{o=0..6}
    raw[4*t2+o, :] @ P'_o where P'_o combines projector rows (CPU prep).
    Computed transposed (d on partitions) via 7 accumulating fp32r matmuls
    against a strided view of rawT.
  * scores[t2, c] = 2*t.c - ||c||^2 (argmax == argmin of distance) via
    fp32r matmuls; the -||c||^2 bias enters PSUM through a K=1 matmul.
    DVE max/max_index give top-8 values + the argmax index (first
    occurrence, matching jnp.argmin).
  * fp32r is ~1e-5-relative per dot; rows whose top-2 score gap is below
    a threshold are exactly re-solved on CPU (~1% of rows) so labels
    match the fp32 reference.
  * masked_feats: copy_predicated overwrite of auged with noise where the
    (CPU-computed, tiny) frame mask is set.
"""

import numpy as np
import ml_dtypes

BF16 = np.dtype(ml_dtypes.bfloat16)

B, T, D = 32, 2048, 80
T2 = 511
CB = 1024
NCORES = 8
BPC = B // NCORES  # batches per core
OFFS = list(range(7))  # frame offsets per t2 window
GAP_THRESH = 2.0  # score units; bf16 err-diff rms ~0.34 -> ~6 sigma

_CACHE: dict = {}


def _cpu_jax_consts():
    """Reproduce the reference's fixed PRNG draws (key 42) on CPU jax."""
    if "noise" in _CACHE:
        return _CACHE["noise"], _CACHE["u"]
    import jax

    cpu = jax.devices("cpu")[0]
    with jax.default_device(cpu):
        import jax.numpy as jnp

        mkey = jax.random.key(42)
        km, kn = jax.random.split(mkey)
        u = np.asarray(jax.random.uniform(km, (B, T2)))
        noise = np.asarray(0.1 * jax.random.normal(kn, (B, T, D), jnp.float32))
    _CACHE["noise"] = noise
    _CACHE["u"] = u
    return noise, u


def _build_program():
    if "nc" in _CACHE:
        return _CACHE["nc"]
    import concourse.bacc as bacc
    from concourse import mybir
    from concourse.tile import TileContext

    f32 = mybir.dt.float32
    bf16 = mybir.dt.bfloat16
    u32 = mybir.dt.uint32

    nc = bacc.Bacc()
    rawT = nc.declare_dram_parameter("rawT", [BPC, 128, 4, 516], bf16, isOutput=False)
    auged = nc.declare_dram_parameter("auged", [BPC, 2048, 80], f32, isOutput=False)
    noise = nc.declare_dram_parameter("noise", [BPC, 2048, 80], f32, isOutput=False)
    i32 = mybir.dt.int32
    selbc = nc.declare_dram_parameter("selbc", [BPC, 128, 16], i32, isOutput=False)
    pproj = nc.declare_dram_parameter("pproj", [128, 7, 512], bf16, isOutput=False)
    cb2t = nc.declare_dram_parameter("cb2t", [128, 4, 1024], bf16, isOutput=False)
    nbias = nc.declare_dram_parameter("nbias", [128, 1024], bf16, isOutput=False)
    ones_d = nc.declare_dram_parameter("ones_d", [128, 128], bf16, isOutput=False)
    masked = nc.declare_dram_parameter("masked", [BPC, 2048, 80], f32, isOutput=True)
    labout = nc.declare_dram_parameter("labout", [BPC, 128, 4, 3], u32, isOutput=True)

    with TileContext(nc) as tc:
        with (
            tc.tile_pool(name="const", bufs=1) as const,
            tc.tile_pool(name="raw", bufs=2) as raw_pool,
            tc.tile_pool(name="tps", bufs=4, space="PSUM") as tpsum_pool,
            tc.tile_pool(name="tsb", bufs=2) as tsb_pool,
            tc.tile_pool(name="sps", bufs=2, space="PSUM") as spsum_pool,
            tc.tile_pool(name="ssb", bufs=3) as ssb_pool,
            tc.tile_pool(name="mx", bufs=4) as mx_pool,
            tc.tile_pool(name="ix", bufs=4) as ix_pool,
            tc.tile_pool(name="io", bufs=2) as io_pool,
            tc.tile_pool(name="sel", bufs=2) as sel_pool,
        ):
            pp_sb = const.tile([128, 7, 512], bf16, name="pp_sb")
            nc.sync.dma_start(out=pp_sb, in_=pproj[:])
            cb_sb = const.tile([128, 4, 1024], bf16, name="cb_sb")
            nb_sb = const.tile([128, 1024], bf16, name="nb_sb")
            ones = const.tile([128, 128], bf16, name="ones")

            for b in range(BPC):
                lab_acc = mx_pool.tile([128, 4, 3], u32, name="lab_acc")
                rawt = raw_pool.tile([128, 4, 516], bf16, name="rawt")
                for ph in range(4):
                    nc.sync.dma_start(out=rawt[:, ph, :], in_=rawT[b, :, ph, :])
                if b == 0:
                    # constants needed only from the scores phase onward
                    nc.sync.dma_start(out=cb_sb, in_=cb2t[:])
                    nc.sync.dma_start(out=nb_sb, in_=nbias[:])
                    nc.sync.dma_start(out=ones, in_=ones_d[:])
                tsb = tsb_pool.tile([128, 4, 512], bf16, name="tsb")
                for kc in range(4):
                    tps = tpsum_pool.tile([128, 512], f32, name="tps")
                    for i, o in enumerate(OFFS):
                        nc.tensor.matmul(
                            tps[:, 0:512],
                            lhsT=pp_sb[:, i, kc * 128 : (kc + 1) * 128],
                            rhs=rawt[:, o % 4, o // 4 : o // 4 + 512],
                            start=(i == 0),
                            stop=(i == len(OFFS) - 1),
                        )
                    nc.scalar.copy(out=tsb[:, kc, :], in_=tps[:, :])

                aug = io_pool.tile([128, 1280], f32, name="aug")
                nc.sync.dma_start(
                    out=aug, in_=auged[b].rearrange("(p j) f -> p (j f)", p=128)
                )
                noi = io_pool.tile([128, 1280], f32, name="noi")
                nc.sync.dma_start(
                    out=noi, in_=noise[b].rearrange("(p j) f -> p (j f)", p=128)
                )
                sel = sel_pool.tile([128, 16], i32, name="sel")
                nc.sync.dma_start(out=sel, in_=selbc[b])
                selb = sel.unsqueeze(2).broadcast_to([128, 16, 80])
                # emit InstCopyPredicated with opt=False so all three APs
                # keep the same 3-D shape (the sim can't merge the step-0
                # broadcast mask while merging the contiguous operands)
                vec = nc.vector
                vec.add_instruction(
                    mybir.InstCopyPredicated(
                        name=f"I-{nc.next_id()}",
                        ins=[
                            vec.lower_ap(selb, opt=False),
                            vec.lower_ap(
                                noi.rearrange("p (j f) -> p j f", f=80), opt=False
                            ),
                        ],
                        outs=[
                            vec.lower_ap(
                                aug.rearrange("p (j f) -> p j f", f=80), opt=False
                            )
                        ],
                    )
                )
                nc.sync.dma_start(
                    out=masked[b].rearrange("(p j) f -> p (j f)", p=128), in_=aug
                )

                for rc in range(4):
                    M = 128
                    r0 = rc * 128
                    sps = spsum_pool.tile([128, 1024], f32, name="sps")
                    for ch in range(2):
                        # K=2 bias pass: psum <- -||c||^2 (bf16 hi+lo)
                        nc.tensor.matmul(
                            sps[:M, ch * 512 : (ch + 1) * 512],
                            lhsT=ones[:, 0:M],
                            rhs=nb_sb[:, ch * 512 : (ch + 1) * 512],
                            start=True,
                            stop=False,
                        )
                        for kc in range(4):
                            nc.tensor.matmul(
                                sps[:M, ch * 512 : (ch + 1) * 512],
                                lhsT=tsb[:, kc, r0 : r0 + M],
                                rhs=cb_sb[:, kc, ch * 512 : (ch + 1) * 512],
                                start=False,
                                stop=(kc == 3),
                            )
                    ssb = ssb_pool.tile([128, 1024], f32, name="ssb")
                    nc.scalar.copy(out=ssb[:M], in_=sps[:M])
                    mx8 = mx_pool.tile([128, 8], f32, name="mx8")
                    nc.vector.max(mx8[:M], ssb[:M])
                    ix = ix_pool.tile([128, 8], u32, name="ix")
                    nc.vector.max_index(ix[:M], mx8[:M], ssb[:M])
                    nc.vector.tensor_copy(lab_acc[:, rc, 0:1], ix[:, 0:1])
                    nc.vector.tensor_copy(
                        lab_acc[:, rc, 1:3], mx8[:, 0:2].bitcast(u32)
                    )
                nc.gpsimd.dma_start(out=labout[b], in_=lab_acc)

    nc.compile()
    _CACHE["nc"] = nc
    return nc


def _prep(raw_feats, auged_feats, length, projector, codebook):
    """CPU-side prep: weight reindexing, PRNG constants, mask, sharding."""
    raw = np.ascontiguousarray(np.asarray(raw_feats, dtype=np.float32))
    auged = np.ascontiguousarray(np.asarray(auged_feats, dtype=np.float32))
    length = np.asarray(length).astype(np.int64)
    proj = np.asarray(projector, dtype=np.float32)
    cb = np.asarray(codebook, dtype=np.float32)

    noise, u = _cpu_jax_consts()

    # combined projector P'_o (o = 2*j2 + j1)
    pp = np.zeros((7, 80, 512), np.float32)
    for j1 in range(3):
        for j2 in range(3):
            pp[2 * j2 + j1] += proj[3 * j1 + j2 :: 9, :]
    pproj = np.zeros((128, 7, 512), BF16)
    pproj[:80] = pp.transpose(1, 0, 2).astype(BF16)

    cb2t = np.ascontiguousarray(
        (2.0 * cb.T).reshape(4, 128, 1024).transpose(1, 0, 2).astype(BF16)
    )  # (128, 4, 1024): [p, k, c] = 2*cb[c, k*128+p]
    nb = (-(cb.astype(np.float64) ** 2).sum(1)).astype(np.float32)
    nbias = np.zeros((128, 1024), BF16)
    nbias[0] = nb.astype(BF16)
    nbias[1] = (nb - nbias[0].astype(np.float32)).astype(BF16)

    ll = ((length - 3) // 2 + 1 - 3) // 2 + 1
    valid = np.arange(T2)[None, :] < ll[:, None]
    masked_steps = (u < 0.1) & valid  # (B, T2)
    mdim = masked_steps.astype(np.float32)

    # frame mask: union of windows 4*t2 + {0..6} over masked steps
    fm = np.zeros((B, T), bool)
    t2r = 4 * np.arange(T2)
    for o in range(7):
        np.logical_or.at(fm, (np.arange(B)[:, None], (t2r + o)[None, :]), masked_steps)
    sel = np.ascontiguousarray(fm.reshape(B, 128, 16).astype(np.int32))

    rawT = np.zeros((B, 128, 4, 516), BF16)
    # [b, f, ph, tau] = raw[b, 4*tau+ph, f]
    rawT[:, :80, :, :512] = (
        raw.reshape(B, 512, 4, 80).transpose(0, 3, 2, 1).astype(BF16)
    )

    in_maps = []
    for m in range(NCORES):
        s = slice(m * BPC, (m + 1) * BPC)
        in_maps.append(
            {
                "rawT": rawT[s],
                "auged": auged[s],
                "noise": noise[s],
                "selbc": sel[s],
                "pproj": pproj,
                "cb2t": cb2t,
                "nbias": nbias,
                "ones_d": _CACHE.setdefault("ones128", _mk_ones()),
            }
        )
    return in_maps, valid, mdim


def _mk_ones():
    o = np.zeros((128, 128), BF16)
    o[0:2] = 1.0
    return o


_STACK_F = np.arange(720) // 9
_STACK_OFF = 2 * (np.arange(720) % 3) + (np.arange(720) // 3) % 3


def _fix_near_ties(lab, gaps, raw, proj, cb):
    """Exactly re-solve rows whose device top-2 score gap is small."""
    bs, t2s = np.nonzero(gaps < GAP_THRESH)
    if len(bs) == 0:
        return 0
    frames = (4 * t2s[:, None] + _STACK_OFF[None, :]).astype(np.int64)  # (F, 720)
    stacked = raw[bs[:, None], frames, _STACK_F[None, :]].astype(np.float64)
    t = stacked @ proj.astype(np.float64)  # (F, 512)
    d = (
        (t * t).sum(1)[:, None]
        - 2.0 * (t @ cb.astype(np.float64).T)
        + (cb.astype(np.float64) ** 2).sum(1)[None, :]
    )
    lab[bs, t2s] = d.argmin(1).astype(np.int64) + 1
    return len(bs)


def kernel(raw_feats, auged_feats, length, projector, codebook):
    from concourse.bass_utils import run_bass_kernel_spmd

    in_maps, valid, mdim = _prep(raw_feats, auged_feats, length, projector, codebook)
    nc = _build_program()

    trace = bool(_CACHE.get("trace", False))
    res = run_bass_kernel_spmd(
        nc, in_maps, list(range(NCORES)), trace=trace, **_CACHE.get("run_kwargs", {})
    )
    _CACHE["last_results"] = res

    masked_feats = np.concatenate(
        [np.asarray(res.results[m]["masked"]) for m in range(NCORES)], axis=0
    )
    labout = np.concatenate(
        [np.asarray(res.results[m]["labout"]) for m in range(NCORES)], axis=0
    )  # (B, 128, 4, 3) uint32: [...,0]=idx, [...,1:3]=top2 scores bits
    lo = np.ascontiguousarray(labout.transpose(0, 2, 1, 3).reshape(B, 512, 3))
    lab = lo[:, :T2, 0].astype(np.int64) + 1
    mxs = lo[:, :, 1:3].view(np.float32)
    gaps = (mxs[:, :T2, 0] - mxs[:, :T2, 1])
    nfix = _fix_near_ties(
        lab,
        gaps,
        np.asarray(raw_feats, dtype=np.float32),
        np.asarray(projector, dtype=np.float32),
        np.asarray(codebook, dtype=np.float32),
    )
    _CACHE["last_nfix"] = nfix

    labels = np.where(valid, lab, 0).astype(np.int32)[None]
    return masked_feats, labels, mdim


# revision 12
# speedup vs baseline: 2.2595x; 1.0421x over previous
"""BestRQ layer (vq_codebook) on 8 TRN2 NeuronCores — data parallel over batch.

Math (reference semantics):
  stacked = double-unfold(raw, k=3,s=2 twice)          (B, 511, 720)
  targets = stacked @ projector                        (B, 511, 512)
  labels  = argmin_c ||targets - codebook[c]||^2 + 1, zeroed past label_lengths
  masked  = fixed-PRNG mask (key 42) on valid steps; frames under masked
            windows replaced by fixed-PRNG noise in auged_feats.

Device decomposition (per core, 4 batch items):
  * double-unfold + projector == strided conv: targets[t2] = sum_{o=0..6}
    raw[4*t2+o, :] @ P'_o where P'_o combines projector rows (CPU prep).
    Computed transposed (d on partitions) via 7 accumulating fp32r matmuls
    against a strided view of rawT.
  * scores[t2, c] = 2*t.c - ||c||^2 (argmax == argmin of distance) via
    fp32r matmuls; the -||c||^2 bias enters PSUM through a K=1 matmul.
    DVE max/max_index give top-8 values + the argmax index (first
    occurrence, matching jnp.argmin).
  * fp32r is ~1e-5-relative per dot; rows whose top-2 score gap is below
    a threshold are exactly re-solved on CPU (~1% of rows) so labels
    match the fp32 reference.
  * masked_feats: copy_predicated overwrite of auged with noise where the
    (CPU-computed, tiny) frame mask is set.
"""

import numpy as np
import ml_dtypes

BF16 = np.dtype(ml_dtypes.bfloat16)

B, T, D = 32, 2048, 80
T2 = 511
CB = 1024
NCORES = 8
BPC = B // NCORES  # batches per core
OFFS = list(range(7))  # frame offsets per t2 window
GAP_THRESH = 2.0  # score units; bf16 err-diff rms ~0.34 -> ~6 sigma

_CACHE: dict = {}


def _cpu_jax_consts():
    """Reproduce the reference's fixed PRNG draws (key 42) on CPU jax."""
    if "noise" in _CACHE:
        return _CACHE["noise"], _CACHE["u"]
    import jax

    cpu = jax.devices("cpu")[0]
    with jax.default_device(cpu):
        import jax.numpy as jnp

        mkey = jax.random.key(42)
        km, kn = jax.random.split(mkey)
        u = np.asarray(jax.random.uniform(km, (B, T2)))
        noise = np.asarray(0.1 * jax.random.normal(kn, (B, T, D), jnp.float32))
    _CACHE["noise"] = noise
    _CACHE["u"] = u
    return noise, u


def _build_program():
    if "nc" in _CACHE:
        return _CACHE["nc"]
    import concourse.bacc as bacc
    from concourse import mybir
    from concourse.tile import TileContext

    f32 = mybir.dt.float32
    bf16 = mybir.dt.bfloat16
    u32 = mybir.dt.uint32

    nc = bacc.Bacc()
    raws = nc.declare_dram_parameter("raws", [BPC, 5, 128, 516], bf16, isOutput=False)
    auged = nc.declare_dram_parameter("auged", [BPC, 2048, 80], f32, isOutput=False)
    noise = nc.declare_dram_parameter("noise", [BPC, 2048, 80], f32, isOutput=False)
    i32 = mybir.dt.int32
    selbc = nc.declare_dram_parameter("selbc", [BPC, 128, 16], i32, isOutput=False)
    pproj = nc.declare_dram_parameter("pproj", [5, 128, 512], bf16, isOutput=False)
    cb2t = nc.declare_dram_parameter("cb2t", [128, 4, 1024], bf16, isOutput=False)
    nbias = nc.declare_dram_parameter("nbias", [128, 1024], bf16, isOutput=False)
    ones_d = nc.declare_dram_parameter("ones_d", [128, 128], bf16, isOutput=False)
    masked = nc.declare_dram_parameter("masked", [BPC, 2048, 80], f32, isOutput=True)
    labout = nc.declare_dram_parameter("labout", [BPC, 128, 4, 3], u32, isOutput=True)

    with TileContext(nc) as tc:
        with (
            tc.tile_pool(name="const", bufs=1) as const,
            tc.tile_pool(name="raw", bufs=2) as raw_pool,
            tc.tile_pool(name="tps", bufs=4, space="PSUM") as tpsum_pool,
            tc.tile_pool(name="tsb", bufs=2) as tsb_pool,
            tc.tile_pool(name="sps", bufs=2, space="PSUM") as spsum_pool,
            tc.tile_pool(name="ssb", bufs=3) as ssb_pool,
            tc.tile_pool(name="mx", bufs=4) as mx_pool,
            tc.tile_pool(name="ix", bufs=4) as ix_pool,
            tc.tile_pool(name="io", bufs=2) as io_pool,
            tc.tile_pool(name="sel", bufs=2) as sel_pool,
        ):
            pp_sb = const.tile([128, 5, 512], bf16, name="pp_sb")
            for k in range(5):
                nc.sync.dma_start(out=pp_sb[:, k, :], in_=pproj[k])
            cb_sb = const.tile([128, 4, 1024], bf16, name="cb_sb")
            nb_sb = const.tile([128, 1024], bf16, name="nb_sb")
            ones = const.tile([128, 128], bf16, name="ones")

            for b in range(BPC):
                lab_acc = mx_pool.tile([128, 4, 3], u32, name="lab_acc")
                rawt = raw_pool.tile([128, 5, 516], bf16, name="rawt")
                for k in range(5):
                    nc.sync.dma_start(out=rawt[:, k, :], in_=raws[b, k])
                if b == 0:
                    # constants needed only from the scores phase onward
                    nc.sync.dma_start(out=cb_sb, in_=cb2t[:])
                    nc.sync.dma_start(out=nb_sb, in_=nbias[:])
                    nc.sync.dma_start(out=ones, in_=ones_d[:])
                tsb = tsb_pool.tile([128, 4, 512], bf16, name="tsb")
                for kc in range(4):
                    tps = tpsum_pool.tile([128, 512], f32, name="tps")
                    for k in range(5):
                        nc.tensor.matmul(
                            tps[:, 0:512],
                            lhsT=pp_sb[:, k, kc * 128 : (kc + 1) * 128],
                            rhs=rawt[:, k, 0:512],
                            start=(k == 0),
                            stop=(k == 4),
                        )
                    nc.scalar.copy(out=tsb[:, kc, :], in_=tps[:, :])

                aug = io_pool.tile([128, 1280], f32, name="aug")
                nc.sync.dma_start(
                    out=aug, in_=auged[b].rearrange("(p j) f -> p (j f)", p=128)
                )
                noi = io_pool.tile([128, 1280], f32, name="noi")
                nc.sync.dma_start(
                    out=noi, in_=noise[b].rearrange("(p j) f -> p (j f)", p=128)
                )
                sel = sel_pool.tile([128, 16], i32, name="sel")
                nc.sync.dma_start(out=sel, in_=selbc[b])
                selb = sel.unsqueeze(2).broadcast_to([128, 16, 80])
                # emit InstCopyPredicated with opt=False so all three APs
                # keep the same 3-D shape (the sim can't merge the step-0
                # broadcast mask while merging the contiguous operands)
                vec = nc.vector
                vec.add_instruction(
                    mybir.InstCopyPredicated(
                        name=f"I-{nc.next_id()}",
                        ins=[
                            vec.lower_ap(selb, opt=False),
                            vec.lower_ap(
                                noi.rearrange("p (j f) -> p j f", f=80), opt=False
                            ),
                        ],
                        outs=[
                            vec.lower_ap(
                                aug.rearrange("p (j f) -> p j f", f=80), opt=False
                            )
                        ],
                    )
                )
                nc.sync.dma_start(
                    out=masked[b].rearrange("(p j) f -> p (j f)", p=128), in_=aug
                )

                for rc in range(4):
                    M = 128
                    r0 = rc * 128
                    sps = spsum_pool.tile([128, 1024], f32, name="sps")
                    ssb = ssb_pool.tile([128, 1024], f32, name="ssb")
                    for ch in range(2):
                        # K=2 bias pass: psum <- -||c||^2 (bf16 hi+lo)
                        nc.tensor.matmul(
                            sps[:M, ch * 512 : (ch + 1) * 512],
                            lhsT=ones[:, 0:M],
                            rhs=nb_sb[:, ch * 512 : (ch + 1) * 512],
                            start=True,
                            stop=False,
                        )
                        for kc in range(4):
                            nc.tensor.matmul(
                                sps[:M, ch * 512 : (ch + 1) * 512],
                                lhsT=tsb[:, kc, r0 : r0 + M],
                                rhs=cb_sb[:, kc, ch * 512 : (ch + 1) * 512],
                                start=False,
                                stop=(kc == 3),
                            )
                        nc.scalar.copy(
                            out=ssb[:M, ch * 512 : (ch + 1) * 512],
                            in_=sps[:M, ch * 512 : (ch + 1) * 512],
                        )
                    mx8 = mx_pool.tile([128, 8], f32, name="mx8")
                    nc.vector.max(mx8[:M], ssb[:M])
                    ix = ix_pool.tile([128, 8], u32, name="ix")
                    nc.vector.max_index(ix[:M], mx8[:M], ssb[:M])
                    nc.vector.tensor_copy(lab_acc[:, rc, 0:1], ix[:, 0:1])
                    nc.vector.tensor_copy(
                        lab_acc[:, rc, 1:3], mx8[:, 0:2].bitcast(u32)
                    )
                nc.gpsimd.dma_start(out=labout[b], in_=lab_acc)

    nc.compile()
    _CACHE["nc"] = nc
    return nc


def _prep(raw_feats, auged_feats, length, projector, codebook):
    """CPU-side prep: weight reindexing, PRNG constants, mask, sharding."""
    raw = np.ascontiguousarray(np.asarray(raw_feats, dtype=np.float32))
    auged = np.ascontiguousarray(np.asarray(auged_feats, dtype=np.float32))
    length = np.asarray(length).astype(np.int64)
    proj = np.asarray(projector, dtype=np.float32)
    cb = np.asarray(codebook, dtype=np.float32)

    noise, u = _cpu_jax_consts()

    # combined projector P'_o (o = 2*j2 + j1)
    pp = np.zeros((7, 80, 512), np.float32)
    for j1 in range(3):
        for j2 in range(3):
            pp[2 * j2 + j1] += proj[3 * j1 + j2 :: 9, :]
    # dense 560-row layout: r = o*80 + f, chunks of 128 rows
    pp560 = np.zeros((640, 512), np.float32)
    pp560[:560] = pp.reshape(560, 512)  # [o*80+f, d] = P'_o[f, d]
    pproj = np.ascontiguousarray(pp560.reshape(5, 128, 512).astype(BF16))

    cb2t = np.ascontiguousarray(
        (2.0 * cb.T).reshape(4, 128, 1024).transpose(1, 0, 2).astype(BF16)
    )  # (128, 4, 1024): [p, k, c] = 2*cb[c, k*128+p]
    nb = (-(cb.astype(np.float64) ** 2).sum(1)).astype(np.float32)
    nbias = np.zeros((128, 1024), BF16)
    nbias[0] = nb.astype(BF16)
    nbias[1] = (nb - nbias[0].astype(np.float32)).astype(BF16)

    ll = ((length - 3) // 2 + 1 - 3) // 2 + 1
    valid = np.arange(T2)[None, :] < ll[:, None]
    masked_steps = (u < 0.1) & valid  # (B, T2)
    mdim = masked_steps.astype(np.float32)

    # frame mask: union of windows 4*t2 + {0..6} over masked steps
    fm = np.zeros((B, T), bool)
    t2r = 4 * np.arange(T2)
    for o in range(7):
        np.logical_or.at(fm, (np.arange(B)[:, None], (t2r + o)[None, :]), masked_steps)
    sel = np.ascontiguousarray(fm.reshape(B, 128, 16).astype(np.int32))

    # stacked560[b, o*80+f, tau] = raw[b, 4*tau+o, f]
    v = raw.reshape(B, 512, 4, 80)  # [b, tau, ph, f]
    raws = np.zeros((B, 640, 516), BF16)
    for o in range(7):
        rows = slice(o * 80, (o + 1) * 80)
        if o < 4:
            raws[:, rows, :512] = v[:, :, o, :].transpose(0, 2, 1).astype(BF16)
        else:
            raws[:, rows, :511] = v[:, 1:, o - 4, :].transpose(0, 2, 1).astype(BF16)
    raws = np.ascontiguousarray(raws.reshape(B, 5, 128, 516))

    in_maps = []
    for m in range(NCORES):
        s = slice(m * BPC, (m + 1) * BPC)
        in_maps.append(
            {
                "raws": raws[s],
                "auged": auged[s],
                "noise": noise[s],
                "selbc": sel[s],
                "pproj": pproj,
                "cb2t": cb2t,
                "nbias": nbias,
                "ones_d": _CACHE.setdefault("ones128", _mk_ones()),
            }
        )
    return in_maps, valid, mdim


def _mk_ones():
    o = np.zeros((128, 128), BF16)
    o[0:2] = 1.0
    return o


_STACK_F = np.arange(720) // 9
_STACK_OFF = 2 * (np.arange(720) % 3) + (np.arange(720) // 3) % 3


def _fix_near_ties(lab, gaps, raw, proj, cb):
    """Exactly re-solve rows whose device top-2 score gap is small."""
    bs, t2s = np.nonzero(gaps < GAP_THRESH)
    if len(bs) == 0:
        return 0
    frames = (4 * t2s[:, None] + _STACK_OFF[None, :]).astype(np.int64)  # (F, 720)
    stacked = raw[bs[:, None], frames, _STACK_F[None, :]].astype(np.float64)
    t = stacked @ proj.astype(np.float64)  # (F, 512)
    d = (
        (t * t).sum(1)[:, None]
        - 2.0 * (t @ cb.astype(np.float64).T)
        + (cb.astype(np.float64) ** 2).sum(1)[None, :]
    )
    lab[bs, t2s] = d.argmin(1).astype(np.int64) + 1
    return len(bs)


def kernel(raw_feats, auged_feats, length, projector, codebook):
    from concourse.bass_utils import run_bass_kernel_spmd

    in_maps, valid, mdim = _prep(raw_feats, auged_feats, length, projector, codebook)
    nc = _build_program()

    trace = bool(_CACHE.get("trace", False))
    res = run_bass_kernel_spmd(
        nc, in_maps, list(range(NCORES)), trace=trace, **_CACHE.get("run_kwargs", {})
    )
    _CACHE["last_results"] = res

    masked_feats = np.concatenate(
        [np.asarray(res.results[m]["masked"]) for m in range(NCORES)], axis=0
    )
    labout = np.concatenate(
        [np.asarray(res.results[m]["labout"]) for m in range(NCORES)], axis=0
    )  # (B, 128, 4, 3) uint32: [...,0]=idx, [...,1:3]=top2 scores bits
    lo = np.ascontiguousarray(labout.transpose(0, 2, 1, 3).reshape(B, 512, 3))
    lab = lo[:, :T2, 0].astype(np.int64) + 1
    mxs = lo[:, :, 1:3].view(np.float32)
    gaps = (mxs[:, :T2, 0] - mxs[:, :T2, 1])
    nfix = _fix_near_ties(
        lab,
        gaps,
        np.asarray(raw_feats, dtype=np.float32),
        np.asarray(projector, dtype=np.float32),
        np.asarray(codebook, dtype=np.float32),
    )
    _CACHE["last_nfix"] = nfix

    labels = np.where(valid, lab, 0).astype(np.int32)[None]
    return masked_feats, labels, mdim


# revision 14
# speedup vs baseline: 2.3563x; 1.0429x over previous
"""BestRQ layer (vq_codebook) on 8 TRN2 NeuronCores — data parallel over batch.

Math (reference semantics):
  stacked = double-unfold(raw, k=3,s=2 twice)          (B, 511, 720)
  targets = stacked @ projector                        (B, 511, 512)
  labels  = argmin_c ||targets - codebook[c]||^2 + 1, zeroed past label_lengths
  masked  = fixed-PRNG mask (key 42) on valid steps; frames under masked
            windows replaced by fixed-PRNG noise in auged_feats.

Device decomposition (per core, 4 batch items):
  * double-unfold + projector: CPU pre-gathers raw into a dense 560-row
    layout stacked560[o*80+f, tau] = raw[4*tau+o, f] (7 window offsets x
    80 feats), zero-padded to 5 K=128 chunks. targetsT (d on partitions)
    = 5 accumulating bf16 matmuls against the combined projector P'.
    All matmuls are zero-padded to K=128: partial-K matmuls don't
    register as PE-busy to the HAM clock governor and the PE gets stuck
    at 1.2 GHz instead of 2.4 GHz.
  * scores[t2, c] = 2*t.c - ||c||^2 (argmax == argmin of distance) via
    bf16 matmuls; the bias enters PSUM through a K=128 matmul whose first
    two rows are the bf16 hi+lo split of -||c||^2 (bias error ~8e-3).
    DVE max/max_index give the top-8 values + argmax index (first
    occurrence, matching jnp.argmin).
  * bf16 score error is ~0.3 rms; rows whose top-2 score gap is under
    GAP_THRESH are exactly re-solved on CPU (~10% of rows) so labels
    match the fp32 reference exactly.
  * masked_feats: copy_predicated overwrite of auged with noise where the
    (CPU-computed, tiny) frame mask is set — bit-exact where() semantics.
"""

import numpy as np
import ml_dtypes

BF16 = np.dtype(ml_dtypes.bfloat16)

B, T, D = 32, 2048, 80
T2 = 511
CB = 1024
NCORES = 8
BPC = B // NCORES  # batches per core
OFFS = list(range(7))  # frame offsets per t2 window
GAP_THRESH = 2.0  # score units; bf16 err-diff rms ~0.34 -> ~6 sigma

_CACHE: dict = {}


def _cpu_jax_consts():
    """Reproduce the reference's fixed PRNG draws (key 42) on CPU jax."""
    if "noise" in _CACHE:
        return _CACHE["noise"], _CACHE["u"]
    import jax

    cpu = jax.devices("cpu")[0]
    with jax.default_device(cpu):
        import jax.numpy as jnp

        mkey = jax.random.key(42)
        km, kn = jax.random.split(mkey)
        u = np.asarray(jax.random.uniform(km, (B, T2)))
        noise = np.asarray(0.1 * jax.random.normal(kn, (B, T, D), jnp.float32))
    _CACHE["noise"] = noise
    _CACHE["u"] = u
    return noise, u


def _build_program():
    if "nc" in _CACHE:
        return _CACHE["nc"]
    import concourse.bacc as bacc
    from concourse import mybir
    from concourse.tile import TileContext

    f32 = mybir.dt.float32
    bf16 = mybir.dt.bfloat16
    u32 = mybir.dt.uint32

    nc = bacc.Bacc()
    raws = nc.declare_dram_parameter("raws", [BPC, 5, 128, 516], bf16, isOutput=False)
    auged = nc.declare_dram_parameter("auged", [BPC, 2048, 80], f32, isOutput=False)
    noise = nc.declare_dram_parameter("noise", [BPC, 2048, 80], f32, isOutput=False)
    i32 = mybir.dt.int32
    selbc = nc.declare_dram_parameter("selbc", [BPC, 128, 16], i32, isOutput=False)
    pproj = nc.declare_dram_parameter("pproj", [5, 128, 512], bf16, isOutput=False)
    cb2t = nc.declare_dram_parameter("cb2t", [128, 4, 1024], bf16, isOutput=False)
    nbias = nc.declare_dram_parameter("nbias", [128, 1024], bf16, isOutput=False)
    ones_d = nc.declare_dram_parameter("ones_d", [128, 128], bf16, isOutput=False)
    masked = nc.declare_dram_parameter("masked", [BPC, 2048, 80], f32, isOutput=True)
    labout = nc.declare_dram_parameter("labout", [BPC, 128, 4, 3], u32, isOutput=True)

    with TileContext(nc) as tc:
        with (
            tc.tile_pool(name="const", bufs=1) as const,
            tc.tile_pool(name="raw", bufs=2) as raw_pool,
            tc.tile_pool(name="tps", bufs=2, space="PSUM") as tpsum_pool,
            tc.tile_pool(name="tsb", bufs=2) as tsb_pool,
            tc.tile_pool(name="sps", bufs=3, space="PSUM") as spsum_pool,
            tc.tile_pool(name="ssb", bufs=4) as ssb_pool,
            tc.tile_pool(name="mx", bufs=6) as mx_pool,
            tc.tile_pool(name="ix", bufs=6) as ix_pool,
            tc.tile_pool(name="io", bufs=2) as io_pool,
            tc.tile_pool(name="sel", bufs=2) as sel_pool,
        ):
            pp_sb = const.tile([128, 5, 512], bf16, name="pp_sb")
            cb_sb = const.tile([128, 4, 1024], bf16, name="cb_sb")
            nb_sb = const.tile([128, 1024], bf16, name="nb_sb")
            ones = const.tile([128, 128], bf16, name="ones")

            for b in range(BPC):
                lab_acc = mx_pool.tile([128, 4, 3], u32, name="lab_acc")
                rawt = raw_pool.tile([128, 5, 516], bf16, name="rawt")
                for k in range(5):
                    if b == 0:
                        nc.sync.dma_start(out=pp_sb[:, k, :], in_=pproj[k])
                    nc.sync.dma_start(out=rawt[:, k, :], in_=raws[b, k])
                if b == 0:
                    # constants needed only from the scores phase onward
                    nc.sync.dma_start(out=cb_sb, in_=cb2t[:])
                    nc.sync.dma_start(out=nb_sb, in_=nbias[:])
                    nc.sync.dma_start(out=ones, in_=ones_d[:])
                tsb = tsb_pool.tile([128, 4, 512], bf16, name="tsb")
                for kc in range(4):
                    tps = tpsum_pool.tile([128, 512], f32, name="tps")
                    for k in range(5):
                        nc.tensor.matmul(
                            tps[:, 0:512],
                            lhsT=pp_sb[:, k, kc * 128 : (kc + 1) * 128],
                            rhs=rawt[:, k, 0:512],
                            start=(k == 0),
                            stop=(k == 4),
                        )
                    nc.scalar.copy(out=tsb[:, kc, :], in_=tps[:, :])

                aug = io_pool.tile([128, 1280], f32, name="aug")
                nc.sync.dma_start(
                    out=aug, in_=auged[b].rearrange("(p j) f -> p (j f)", p=128)
                )
                noi = io_pool.tile([128, 1280], f32, name="noi")
                nc.sync.dma_start(
                    out=noi, in_=noise[b].rearrange("(p j) f -> p (j f)", p=128)
                )
                sel = sel_pool.tile([128, 16], i32, name="sel")
                nc.sync.dma_start(out=sel, in_=selbc[b])
                selb = sel.unsqueeze(2).broadcast_to([128, 16, 80])
                # emit InstCopyPredicated with opt=False so all three APs
                # keep the same 3-D shape (the sim can't merge the step-0
                # broadcast mask while merging the contiguous operands)
                vec = nc.vector
                vec.add_instruction(
                    mybir.InstCopyPredicated(
                        name=f"I-{nc.next_id()}",
                        ins=[
                            vec.lower_ap(selb, opt=False),
                            vec.lower_ap(
                                noi.rearrange("p (j f) -> p j f", f=80), opt=False
                            ),
                        ],
                        outs=[
                            vec.lower_ap(
                                aug.rearrange("p (j f) -> p j f", f=80), opt=False
                            )
                        ],
                    )
                )
                nc.sync.dma_start(
                    out=masked[b].rearrange("(p j) f -> p (j f)", p=128), in_=aug
                )

                for rc in range(4):
                    M = 128
                    r0 = rc * 128
                    sps = spsum_pool.tile([128, 1024], f32, name="sps")
                    ssb = ssb_pool.tile([128, 1024], f32, name="ssb")
                    for ch in range(2):
                        # K=2 bias pass: psum <- -||c||^2 (bf16 hi+lo)
                        nc.tensor.matmul(
                            sps[:M, ch * 512 : (ch + 1) * 512],
                            lhsT=ones[:, 0:M],
                            rhs=nb_sb[:, ch * 512 : (ch + 1) * 512],
                            start=True,
                            stop=False,
                        )
                        for kc in range(4):
                            nc.tensor.matmul(
                                sps[:M, ch * 512 : (ch + 1) * 512],
                                lhsT=tsb[:, kc, r0 : r0 + M],
                                rhs=cb_sb[:, kc, ch * 512 : (ch + 1) * 512],
                                start=False,
                                stop=(kc == 3),
                            )
                        nc.scalar.copy(
                            out=ssb[:M, ch * 512 : (ch + 1) * 512],
                            in_=sps[:M, ch * 512 : (ch + 1) * 512],
                        )
                    mx8 = mx_pool.tile([128, 8], f32, name="mx8")
                    nc.vector.max(mx8[:M], ssb[:M])
                    ix = ix_pool.tile([128, 8], u32, name="ix")
                    nc.vector.max_index(ix[:M], mx8[:M], ssb[:M])
                    nc.vector.tensor_copy(lab_acc[:, rc, 0:1], ix[:, 0:1])
                    nc.vector.tensor_copy(
                        lab_acc[:, rc, 1:3], mx8[:, 0:2].bitcast(u32)
                    )
                nc.gpsimd.dma_start(out=labout[b], in_=lab_acc)

    nc.compile()
    _CACHE["nc"] = nc
    return nc


def _prep(raw_feats, auged_feats, length, projector, codebook):
    """CPU-side prep: weight reindexing, PRNG constants, mask, sharding."""
    raw = np.ascontiguousarray(np.asarray(raw_feats, dtype=np.float32))
    auged = np.ascontiguousarray(np.asarray(auged_feats, dtype=np.float32))
    length = np.asarray(length).astype(np.int64)
    proj = np.asarray(projector, dtype=np.float32)
    cb = np.asarray(codebook, dtype=np.float32)

    noise, u = _cpu_jax_consts()

    # combined projector P'_o (o = 2*j2 + j1)
    pp = np.zeros((7, 80, 512), np.float32)
    for j1 in range(3):
        for j2 in range(3):
            pp[2 * j2 + j1] += proj[3 * j1 + j2 :: 9, :]
    # dense 560-row layout: r = o*80 + f, chunks of 128 rows
    pp560 = np.zeros((640, 512), np.float32)
    pp560[:560] = pp.reshape(560, 512)  # [o*80+f, d] = P'_o[f, d]
    pproj = np.ascontiguousarray(pp560.reshape(5, 128, 512).astype(BF16))

    cb2t = np.ascontiguousarray(
        (2.0 * cb.T).reshape(4, 128, 1024).transpose(1, 0, 2).astype(BF16)
    )  # (128, 4, 1024): [p, k, c] = 2*cb[c, k*128+p]
    nb = (-(cb.astype(np.float64) ** 2).sum(1)).astype(np.float32)
    nbias = np.zeros((128, 1024), BF16)
    nbias[0] = nb.astype(BF16)
    nbias[1] = (nb - nbias[0].astype(np.float32)).astype(BF16)

    ll = ((length - 3) // 2 + 1 - 3) // 2 + 1
    valid = np.arange(T2)[None, :] < ll[:, None]
    masked_steps = (u < 0.1) & valid  # (B, T2)
    mdim = masked_steps.astype(np.float32)

    # frame mask: union of windows 4*t2 + {0..6} over masked steps
    fm = np.zeros((B, T), bool)
    t2r = 4 * np.arange(T2)
    for o in range(7):
        np.logical_or.at(fm, (np.arange(B)[:, None], (t2r + o)[None, :]), masked_steps)
    sel = np.ascontiguousarray(fm.reshape(B, 128, 16).astype(np.int32))

    # stacked560[b, o*80+f, tau] = raw[b, 4*tau+o, f]
    v = raw.reshape(B, 512, 4, 80)  # [b, tau, ph, f]
    raws = np.zeros((B, 640, 516), BF16)
    for o in range(7):
        rows = slice(o * 80, (o + 1) * 80)
        if o < 4:
            raws[:, rows, :512] = v[:, :, o, :].transpose(0, 2, 1).astype(BF16)
        else:
            raws[:, rows, :511] = v[:, 1:, o - 4, :].transpose(0, 2, 1).astype(BF16)
    raws = np.ascontiguousarray(raws.reshape(B, 5, 128, 516))

    in_maps = []
    for m in range(NCORES):
        s = slice(m * BPC, (m + 1) * BPC)
        in_maps.append(
            {
                "raws": raws[s],
                "auged": auged[s],
                "noise": noise[s],
                "selbc": sel[s],
                "pproj": pproj,
                "cb2t": cb2t,
                "nbias": nbias,
                "ones_d": _CACHE.setdefault("ones128", _mk_ones()),
            }
        )
    return in_maps, valid, mdim


def _mk_ones():
    o = np.zeros((128, 128), BF16)
    o[0:2] = 1.0
    return o


_STACK_F = np.arange(720) // 9
_STACK_OFF = 2 * (np.arange(720) % 3) + (np.arange(720) // 3) % 3


def _fix_near_ties(lab, gaps, raw, proj, cb):
    """Exactly re-solve rows whose device top-2 score gap is small."""
    bs, t2s = np.nonzero(gaps < GAP_THRESH)
    if len(bs) == 0:
        return 0
    frames = (4 * t2s[:, None] + _STACK_OFF[None, :]).astype(np.int64)  # (F, 720)
    stacked = raw[bs[:, None], frames, _STACK_F[None, :]].astype(np.float64)
    t = stacked @ proj.astype(np.float64)  # (F, 512)
    d = (
        (t * t).sum(1)[:, None]
        - 2.0 * (t @ cb.astype(np.float64).T)
        + (cb.astype(np.float64) ** 2).sum(1)[None, :]
    )
    lab[bs, t2s] = d.argmin(1).astype(np.int64) + 1
    return len(bs)


def kernel(raw_feats, auged_feats, length, projector, codebook):
    from concourse.bass_utils import run_bass_kernel_spmd

    in_maps, valid, mdim = _prep(raw_feats, auged_feats, length, projector, codebook)
    nc = _build_program()

    trace = bool(_CACHE.get("trace", False))
    res = run_bass_kernel_spmd(
        nc, in_maps, list(range(NCORES)), trace=trace, **_CACHE.get("run_kwargs", {})
    )
    _CACHE["last_results"] = res

    masked_feats = np.concatenate(
        [np.asarray(res.results[m]["masked"]) for m in range(NCORES)], axis=0
    )
    labout = np.concatenate(
        [np.asarray(res.results[m]["labout"]) for m in range(NCORES)], axis=0
    )  # (B, 128, 4, 3) uint32: [...,0]=idx, [...,1:3]=top2 scores bits
    lo = np.ascontiguousarray(labout.transpose(0, 2, 1, 3).reshape(B, 512, 3))
    lab = lo[:, :T2, 0].astype(np.int64) + 1
    mxs = lo[:, :, 1:3].view(np.float32)
    gaps = (mxs[:, :T2, 0] - mxs[:, :T2, 1])
    nfix = _fix_near_ties(
        lab,
        gaps,
        np.asarray(raw_feats, dtype=np.float32),
        np.asarray(projector, dtype=np.float32),
        np.asarray(codebook, dtype=np.float32),
    )
    _CACHE["last_nfix"] = nfix

    labels = np.where(valid, lab, 0).astype(np.int32)[None]
    return masked_feats, labels, mdim
